# revision 31
# baseline (speedup 1.0000x reference)
# Trainium2 Bass kernel for nn_BasicTransformerBlockST (spatio-temporal
# transformer block: windowed spatial self-attention, two temporal
# self-attentions with relative-position bias + causal mask, cross-attention
# to a text context, and a GEGLU feed-forward).
#
# Sharding: data-parallel over the 128 (b, nh, nw) spatial windows -> 16
# windows x 4096 tokens per core; every stage (window attn / temporal attn /
# cross attn / FF) is closed under this shard, so no collectives are needed.
#
# Per-core layout: activations channel-major xT [C=320, ntok] resident in SBUF
# as [128, 3, ntok]; token order (window, spatial, t) makes temporal sequences
# contiguous 16-token runs. Softmax runs in transposed score space S^T[k, q]
# without max subtraction (logits are O(1)); the normalizer comes from an
# appended ones-column in V and is applied post-attention. Masked entries get
# -1e5 before exp and underflow to exactly 0, which makes the block-diagonal
# batched temporal attention exact. LayerNorm is folded: gamma/beta fold into
# the projection weights; the -mu*rstd and beta terms ride along as augmented
# contraction rows.
import numpy as np
import ml_dtypes

import concourse.bass as bass
import concourse.tile as tile
from concourse import bacc, mybir
from concourse.bass_utils import run_bass_kernel_spmd

F32 = mybir.dt.float32
F32R = mybir.dt.float32r
BF16 = mybir.dt.bfloat16
AF = mybir.ActivationFunctionType
ALU = mybir.AluOpType

D, CTX_DIM, HEADS, DH, T_LEN, WS, MAXREL, FF = 320, 768, 8, 40, 16, 4, 16, 1280
B, H, W = 2, 32, 32
NH = H // WS
NWIN = B * NH * NH          # 128 windows total
NCORES = 8
SEQ_TOK = T_LEN * WS * WS   # 256 tokens per window
SCALE = DH ** -0.5
NEG = -1e5
NCTX = 77
EPS = 1e-5

bfdt = ml_dtypes.bfloat16


# ----------------------------------------------------------------------------
# host-side data prep
# ----------------------------------------------------------------------------

def shard_x(x, win_per_core):
    xr = np.asarray(x, np.float32).reshape(B, D, T_LEN, NH, WS, NH, WS)
    xr = xr.transpose(0, 3, 5, 1, 4, 6, 2)          # B nh nw C wh ww T
    xr = xr.reshape(NWIN, D, WS * WS * T_LEN)       # win C (s t)
    ncore = NWIN // win_per_core
    xr = xr.reshape(ncore, win_per_core, D, WS * WS * T_LEN)
    xr = xr.transpose(0, 2, 1, 3).reshape(ncore, D, win_per_core * WS * WS * T_LEN)
    return np.ascontiguousarray(xr)


def unshard_x(shards, win_per_core):
    ncore = NWIN // win_per_core
    xr = shards.reshape(ncore, D, win_per_core, WS * WS * T_LEN).transpose(0, 2, 1, 3)
    xr = xr.reshape(B, NH, NH, D, WS, WS, T_LEN)
    xr = xr.transpose(0, 3, 6, 1, 4, 2, 5)          # B C T nh wh nw ww
    return np.ascontiguousarray(xr.reshape(B, D, T_LEN, H, W))


def _cmajor(a, rows):
    """[rows_logical<=rows, cols] -> [128, rows/128, cols], zero padded."""
    out = np.zeros((rows, a.shape[1]), np.float32)
    out[: a.shape[0]] = a
    return np.ascontiguousarray(
        out.reshape(rows // 128, 128, a.shape[1]).transpose(1, 0, 2))


VDIM = DH + 1    # 41 per-head value columns; slot 32 is the ones column


def vslot(c):
    """map v-slot index c in [0,41) to head dim, or None for the ones slot."""
    if c == 32:
        return None
    return c if c < 32 else c - 1


def pad_v_cols(Wv):
    """[cin, 320] -> [cin, 328]: per-head 41 columns; slot 32 left zero
    (filled with ones on device for the softmax-denominator trick)."""
    cin = Wv.shape[0]
    out = np.zeros((cin, HEADS * VDIM), np.float32)
    for h in range(HEADS):
        for c in range(VDIM):
            d = vslot(c)
            if d is not None:
                out[:, h * VDIM + c] = Wv[:, h * DH + d]
    return out


def pad_head_cols(Wx):
    """[cin, 320] -> [cin, 512]: head h cols at h*64+[0,40), zeros between."""
    out = np.zeros((Wx.shape[0], 512), np.float32)
    for h in range(HEADS):
        out[:, h * 64: h * 64 + 40] = Wx[:, h * 40: (h + 1) * 40]
    return out


def prep_proj_w(Wraw, gamma, beta, extra_bias=None, pad_heads=False):
    """Augmented c-major projection weight [128, 3, dout]:
    rows 0..320 = W*gamma[:,None]; row (2,64) = colsum (pairs with -mu*rstd);
    row (2,65) = beta@Wg (+extra_bias)."""
    Wg = np.asarray(Wraw, np.float32) * np.asarray(gamma, np.float32)[:, None]
    if pad_heads:
        Wg = pad_head_cols(Wg)
    out = np.zeros((384, Wg.shape[1]), np.float32)
    out[:320] = Wg
    out[256 + 64] = Wg.sum(0)
    out[256 + 65] = np.asarray(beta, np.float32) @ Wg
    if extra_bias is not None:
        out[256 + 65] += np.asarray(extra_bias, np.float32)
    return _cmajor(out, 384)


def prep_wo(Wo, bo):
    """[320, 320] -> lhsT [128, 4, 320]: head h rows at h*64+c for v-slot c
    (zero at the sum slot c=32); bias rides at plane-3 row 105 (a pad row
    that the device keeps at constant 1.0)."""
    out = np.zeros((512, 320), np.float32)
    Wo = np.asarray(Wo, np.float32)
    for h in range(HEADS):
        for c in range(VDIM):
            d = vslot(c)
            if d is not None:
                out[h * 64 + c] = Wo[h * DH + d]
    out[3 * 128 + 105] = np.asarray(bo, np.float32)
    return _cmajor(out, 512)


def prep_tabq(table):
    """relk [33, 40] -> tabQ [128, 256]: col (i*16+J) holds table[J-i+16] in
    rows 0..40 AND a copy in rows 64..104 (so lhsT base matches q's base)."""
    out = np.zeros((128, 256), np.float32)
    t = np.asarray(table, np.float32)
    for i in range(T_LEN):
        for J in range(T_LEN):
            out[:40, i * 16 + J] = t[J - i + MAXREL]
            out[64:104, i * 16 + J] = t[J - i + MAXREL]
    return out


def prep_tvrep(table):
    """relv [33, 40] -> tvrep [128, 16*41]: for query pos i, col i*41+c
    (v-slot c; zero at c=32) row (s*16+j) holds table[j-i+16, dim(c)]."""
    out = np.zeros((128, T_LEN * VDIM), np.float32)
    t = np.asarray(table, np.float32)
    for i in range(T_LEN):
        for s in range(8):
            for j in range(T_LEN):
                for c in range(VDIM):
                    d = vslot(c)
                    if d is not None:
                        out[s * 16 + j, i * VDIM + c] = t[j - i + MAXREL, d]
    return out


def prep_kaug():
    """constant selector [128, 128]: rows b+J (for each base b in
    0/32/64/96) one at cols (s*16+J)."""
    out = np.zeros((128, 128), np.float32)
    for base in (0, 32, 64, 96):
        for s in range(8):
            for J in range(T_LEN):
                out[base + J, s * 16 + J] = 1.0
    return out


def prep_mask():
    """additive [128, 128]: row (s,j), col (s',i): 0 iff s==s' and j<=i."""
    m = np.full((128, 128), NEG, np.float32)
    for s in range(8):
        for j in range(T_LEN):
            m[s * 16 + j, s * 16 + j: (s + 1) * 16] = 0.0
    return m


def prep_selbc():
    """[8, 4, 128]: row h, plane pt: ones over head h's 64-row block."""
    out = np.zeros((8, 4, 128), np.float32)
    for h in range(8):
        out[h, h // 2, 64 * (h % 2): 64 * (h % 2) + 64] = 1.0
    return out


def prep_weights(wd):
    t = {}

    def bfc(x):
        return np.ascontiguousarray(np.asarray(x, np.float32).astype(bfdt))

    for nm, g, b in (('a1', wd['ln1_g'], wd['ln1_b']),
                     ('t1', wd['ln4_g'], wd['ln4_b']),
                     ('t2', wd['ln5_g'], wd['ln5_b'])):
        t[f'{nm}_wq'] = bfc(prep_proj_w(wd[f'{nm}_Wq'], g, b, pad_heads=True))
        t[f'{nm}_wk'] = bfc(prep_proj_w(wd[f'{nm}_Wk'], g, b, pad_heads=True))
        Wvp = pad_v_cols(np.asarray(wd[f'{nm}_Wv'], np.float32))
        t[f'{nm}_wv'] = bfc(prep_proj_w(Wvp, g, b))
        t[f'{nm}_wo'] = bfc(prep_wo(wd[f'{nm}_Wo'], wd[f'{nm}_bo']))
    t['a2_wq'] = bfc(prep_proj_w(wd['a2_Wq'], wd['ln2_g'], wd['ln2_b'],
                                 pad_heads=True))
    t['a2_wk'] = bfc(_cmajor(pad_head_cols(np.asarray(wd['a2_Wk'], np.float32)), 768))
    t['a2_wv'] = bfc(_cmajor(pad_v_cols(np.asarray(wd['a2_Wv'], np.float32)), 768))
    t['a2_wo'] = bfc(prep_wo(wd['a2_Wo'], wd['a2_bo']))
    t['ff_w1'] = bfc(prep_proj_w(wd['ff_W1'], wd['ln3_g'], wd['ln3_b'],
                                 extra_bias=wd['ff_b1']))
    W2aug = np.zeros((1408, 320), np.float32)
    W2aug[:1280] = np.asarray(wd['ff_W2'], np.float32)
    W2aug[1280] = np.asarray(wd['ff_b2'], np.float32)
    t['ff_w2'] = bfc(_cmajor(W2aug, 1408))
    t['t1_tabq'] = bfc(prep_tabq(wd['t1_relk']))
    t['t2_tabq'] = bfc(prep_tabq(wd['t2_relk']))
    t['t1_tvrep'] = bfc(prep_tvrep(wd['t1_relv']))
    t['t2_tvrep'] = bfc(prep_tvrep(wd['t2_relv']))
    t['kaug'] = bfc(prep_kaug())
    t['mask4'] = np.ascontiguousarray(np.tile(prep_mask(), (1, 4)))
    t['selbc'] = bfc(prep_selbc())
    return t


WEIGHT_SHAPES = {}
for _s in ('a1', 't1', 't2'):
    WEIGHT_SHAPES.update({f'{_s}_wq': ([128, 3, 512], BF16),
                          f'{_s}_wk': ([128, 3, 512], BF16),
                          f'{_s}_wv': ([128, 3, 328], BF16),
                          f'{_s}_wo': ([128, 4, 320], BF16)})
WEIGHT_SHAPES.update({
    'a2_wq': ([128, 3, 512], BF16), 'a2_wk': ([128, 6, 512], BF16),
    'a2_wv': ([128, 6, 328], BF16), 'a2_wo': ([128, 4, 320], BF16),
    'ff_w1': ([128, 3, 2560], BF16), 'ff_w2': ([128, 11, 320], BF16),
    't1_tabq': ([128, 256], BF16), 't2_tabq': ([128, 256], BF16),
    't1_tvrep': ([128, 656], BF16), 't2_tvrep': ([128, 656], BF16),
    'kaug': ([128, 128], BF16), 'mask4': ([128, 512], F32),
    'selbc': ([8, 4, 128], BF16),
})

STAGE_WEIGHTS = {
    'a1': ['a1_wq', 'a1_wk', 'a1_wv', 'a1_wo'],
    't1': ['t1_wq', 't1_wk', 't1_wv', 't1_wo', 't1_tabq', 't1_tvrep'],
    'a2': ['a2_wq', 'a2_wk', 'a2_wv', 'a2_wo'],
    't2': ['t2_wq', 't2_wk', 't2_wv', 't2_wo', 't2_tabq', 't2_tvrep'],
    'ff': ['ff_w1', 'ff_w2'],
}


# ----------------------------------------------------------------------------
# device kernel builder
# ----------------------------------------------------------------------------

def hrow(h):
    """(ptile, row0) of head h in the head-padded 512-row q/k layout."""
    return h // 2, (h % 2) * 64


def mkap(t, extra_off, dims):
    return bass.AP(tensor=t.tensor, offset=t.offset + extra_off, ap=[list(d) for d in dims])


class Builder:
    def __init__(self, nwin=16, chunk_win=4, stages=('a1', 't1', 'a2', 't2', 'ff'),
                 sim_gelu=False):
        self.sim_gelu = sim_gelu
        self.nwin = nwin
        self.ntok = nwin * SEQ_TOK
        self.chunk = min(chunk_win * SEQ_TOK, self.ntok)   # tokens per chunk
        self.stages = stages

    def build(self, num_devices=1):
        nc = bacc.Bacc("TRN2", target_bir_lowering=False, debug=False,
                       num_devices=num_devices)
        self.nc = nc
        dram = {}
        dram['xT'] = nc.declare_dram_parameter('xT', [128, 3, self.ntok], F32,
                                               isOutput=False)
        dram['ctxT'] = nc.declare_dram_parameter('ctxT', [128, 6, NCTX], F32,
                                                 isOutput=False)
        for nm, (shp, dt) in WEIGHT_SHAPES.items():
            dram[nm] = nc.declare_dram_parameter(nm, list(shp), dt, isOutput=False)
        out_yT = nc.declare_dram_parameter('yT', [128, 3, self.ntok], F32,
                                           isOutput=True)
        if getattr(self, 'debug_taps', False):
            dram['dbg_q'] = nc.declare_dram_parameter(
                'dbg_q', [128, 4, self.chunk], BF16, isOutput=True)
            dram['dbg_n'] = nc.declare_dram_parameter(
                'dbg_n', [128, 3, 512], BF16, isOutput=True)
            dram['dbg_cb'] = nc.declare_dram_parameter(
                'dbg_cb', [128, 5, 512], BF16, isOutput=True)
            dram['dbg_s'] = nc.declare_dram_parameter(
                'dbg_s', [128, 4, 512], F32, isOutput=True)
        self.dram = dram
        with tile.TileContext(nc) as tc:
            self.tc = tc
            self._emit(out_yT)
        nc.compile()
        return nc

    # ---------------- helpers ----------------
    def _emit(self, out_yT):
        from contextlib import ExitStack
        nc, tc = self.nc, self.tc
        with ExitStack() as ctx:
            resid = ctx.enter_context(tc.tile_pool(name="resid", bufs=1))
            consts = ctx.enter_context(tc.tile_pool(name="consts", bufs=1))

            xT = resid.tile([128, 3, self.ntok], F32)
            nc.sync.dma_start(out=xT, in_=self.dram['xT'][:])
            self.xT = xT

            self.ones_col = consts.tile([128, 1], F32, name="onescol")
            nc.vector.memset(self.ones_col, 1.0)
            self.ones_col_bf = consts.tile([128, 1], BF16, name="onescolbf")
            nc.vector.memset(self.ones_col_bf, 1.0)
            self.eps_col = consts.tile([128, 1], F32, name="epscol")
            nc.vector.memset(self.eps_col, EPS)
            self.ones_row_bf = consts.tile([1, 2048], BF16, name="onesrowbf")
            nc.vector.memset(self.ones_row_bf, 1.0)
            self.mask4 = consts.tile([128, 512], F32, name="mask4c")
            nc.sync.dma_start(out=self.mask4, in_=self.dram['mask4'][:])
            self.kaug = consts.tile([128, 128], BF16, name="kaugc")
            nc.sync.dma_start(out=self.kaug, in_=self.dram['kaug'][:])
            self.selbc = consts.tile([8, 4, 128], BF16, name="selbcc")
            nc.sync.dma_start(out=self.selbc, in_=self.dram['selbc'][:])
            # context stays resident (tiny)
            self.ctxT = consts.tile([128, 6, NCTX], F32, name="ctxTc")
            nc.sync.dma_start(out=self.ctxT, in_=self.dram['ctxT'][:])

            for st in self.stages:
                with ExitStack() as sctx:
                    wpool = sctx.enter_context(
                        tc.tile_pool(name=f"w_{st}", bufs=1))
                    w = {}
                    for nm in STAGE_WEIGHTS[st]:
                        shp, dt = WEIGHT_SHAPES[nm]
                        w[nm] = wpool.tile(list(shp), dt, name=f"sb_{nm}")
                        nc.sync.dma_start(out=w[nm], in_=self.dram[nm][:])
                    if st == 'a1':
                        self.stage_window(sctx, w['a1_wq'], w['a1_wk'],
                                          w['a1_wv'], w['a1_wo'])
                    elif st in ('t1', 't2'):
                        self.stage_temporal(sctx, w[f'{st}_wq'], w[f'{st}_wk'],
                                            w[f'{st}_wv'], w[f'{st}_wo'],
                                            w[f'{st}_tabq'], w[f'{st}_tvrep'])
                    elif st == 'a2':
                        self.stage_cross(sctx, w['a2_wq'], w['a2_wk'],
                                         w['a2_wv'], w['a2_wo'])
                    elif st == 'ff':
                        self.stage_ff(sctx, w['ff_w1'], w['ff_w2'])

            nc.sync.dma_start(out=out_yT[:], in_=xT)

    def ln_stage(self, tok0, ncols, pools):
        """nhat [128, 3, ncols] bf16: rows 0..320 = (x-mu)*rstd;
        row (2,64) = -mu*rstd; row (2,65) = 1.0."""
        nc = self.nc
        xT = self.xT
        stat_psum = pools['stat_psum'].tile([33, 512], F32, tag="statp")
        sq = pools['ln'].tile([128, 3, 512], BF16, tag="lnsq")
        xb = pools['ln'].tile([128, 3, 512], BF16, tag="lnxb")
        cols = slice(tok0, tok0 + ncols)
        # single 3-plane ops: plane-2 pad rows are zero in xT, so summing the
        # full 128 rows of every plane changes nothing
        nc.vector.tensor_copy(out=xb[:, :, :ncols], in_=xT[:, :, cols])
        nc.scalar.square(sq[:, :, :ncols], xT[:, :, cols])
        for k in range(3):
            nc.tensor.matmul(stat_psum[0:1, :ncols],
                             self.ones_col_bf[:128],
                             xb[:, k, :ncols],
                             start=(k == 0), stop=(k == 2))
            nc.tensor.matmul(stat_psum[32:33, :ncols],
                             self.ones_col_bf[:128],
                             sq[:, k, :ncols],
                             start=(k == 0), stop=(k == 2))
        # single-row stat tiles, all at partition 0 (walrus requires equal
        # SBUF start partitions within one TensorTensor op)
        st_mu = pools['ln'].tile([1, 512], F32, tag="lnmu")
        st_ex2 = pools['ln'].tile([1, 512], F32, tag="lnex2")
        st_rstd = pools['ln'].tile([1, 512], F32, tag="lnrstd")
        st_murstd = pools['ln'].tile([1, 512], F32, tag="lnmurstd")
        nc.vector.tensor_scalar_mul(st_mu[:, :ncols], stat_psum[0:1, :ncols], 1.0 / D)
        nc.vector.tensor_scalar_mul(st_ex2[:, :ncols], stat_psum[32:33, :ncols], 1.0 / D)
        nc.vector.tensor_tensor(out=st_rstd[:, :ncols], in0=st_mu[:, :ncols],
                                in1=st_mu[:, :ncols], op=ALU.mult)
        nc.vector.tensor_tensor(out=st_rstd[:, :ncols], in0=st_ex2[:, :ncols],
                                in1=st_rstd[:, :ncols], op=ALU.subtract)
        nc.scalar.activation(st_rstd[:, :ncols], st_rstd[:, :ncols], AF.Sqrt,
                             bias=self.eps_col[:1])
        st_rscr = pools['ln'].tile([1, 512], F32, tag="lnrscr")
        nc.vector.reciprocal_approx_accurate(st_rstd[:, :ncols],
                                             st_rstd[:, :ncols],
                                             st_rscr[:, :ncols])
        nc.vector.tensor_tensor(out=st_murstd[:, :ncols], in0=st_mu[:, :ncols],
                                in1=st_rstd[:, :ncols], op=ALU.mult)
        nc.vector.tensor_scalar_mul(st_murstd[:, :ncols], st_murstd[:, :ncols], -1.0)
        rstd_b = pools['ln'].tile([128, 512], F32, tag="lnrstdb")
        nc.gpsimd.partition_broadcast(rstd_b[:, :ncols], st_rstd[:, :ncols])
        nhat = pools['nhat'].tile([128, 3, 512], BF16, tag="nhat")
        for k in range(3):
            rows = 128 if k < 2 else 64
            nc.vector.tensor_tensor(out=nhat[:rows, k, :ncols],
                                    in0=xT[:rows, k, cols],
                                    in1=rstd_b[:rows, :ncols], op=ALU.mult)
        nhat_mr = pools['ln'].tile([1, 512], BF16, tag="lnmrbf")
        nc.vector.tensor_copy(out=nhat_mr[:, :ncols], in_=st_murstd[:, :ncols])
        # aug rows via DMA (arbitrary partition starts are DMA-only)
        nc.sync.dma_start(out=nhat[64:65, 2, :ncols], in_=nhat_mr[:1, :ncols])
        nc.sync.dma_start(out=nhat[65:66, 2, :ncols],
                          in_=self.ones_row_bf[:1, :ncols])
        return nhat

    KS_AUG = ((0, 128), (1, 128), (2, 66))
    KS_NOAUG = ((0, 128), (1, 128), (2, 64))

    def qkv_chunk(self, c0, pools, w_q, w_k, w_v, qT, kT, vP):
        """LN + q/k/v for tokens [c0, c0+chunk): qT,kT [128,4,chunk] bf16
        c-major head-padded; vP [128, chunk/128, 328] bf16 token-major."""
        nc = self.nc
        ntt = self.chunk // 512
        for n in range(ntt):
            tok0 = c0 + n * 512
            nhat = self.ln_stage(tok0, 512, pools)
            if getattr(self, 'debug_taps', False) and tok0 == 0:
                nc.sync.dma_start(out=self.dram['dbg_n'][:], in_=nhat)
            for wsb, dst, eng in ((w_q, qT, 'act'), (w_k, kT, 'dve')):
                for mt in range(4):
                    ps = pools['psum'].tile([128, 512], F32, tag=f"proj{mt % 2}")
                    for ki, (k, rows) in enumerate(self.KS_AUG):
                        nc.tensor.matmul(ps,
                                         wsb[:rows, k, mt * 128: mt * 128 + 128],
                                         nhat[:rows, k, :],
                                         start=(ki == 0), stop=(ki == 2))
                    dcols = slice(n * 512, (n + 1) * 512)
                    if eng == 'act':
                        nc.scalar.activation(dst[:, mt, dcols], ps, AF.Copy)
                    else:
                        nc.vector.tensor_copy(out=dst[:, mt, dcols], in_=ps)
            for m in range(4):
                ps = pools['psum'].tile([128, 512], F32, tag=f"proj{m % 2}")
                for ki, (k, rows) in enumerate(self.KS_AUG):
                    nc.tensor.matmul(ps[:, :328],
                                     nhat[:rows, k, m * 128:(m + 1) * 128],
                                     w_v[:rows, k, :328],
                                     start=(ki == 0), stop=(ki == 2))
                blk = n * 4 + m
                if m % 2 == 0:
                    nc.scalar.activation(vP[:, blk, :328], ps[:, :328], AF.Copy)
                else:
                    nc.vector.tensor_copy(out=vP[:, blk, :328], in_=ps[:, :328])
        # ones columns: vP[:, :, 32::41]
        onescols = mkap(vP, 32, [[vP.ap[0][0], 128],
                                 [328, self.chunk // 128], [41, 8]])
        nc.gpsimd.memset(onescols, 1.0)

    def finish_heads(self, chunkbuf, srows8, pools, n512):
        """normalize chunkbuf[:, :4, cs] by per-head recips.
        srows8 [8, >=512] f32: head h sum at row h. One reciprocal over the
        8 used rows, then a K=8 selector matmul broadcasts each head's recip
        across its 64-row block, one plane at a time."""
        nc = self.nc
        cs = slice(n512 * 512, (n512 + 1) * 512)
        recf = pools['attn'].tile([8, 512], F32, tag="recipf")
        nc.vector.reciprocal_approx_fast(recf, srows8[:, cs])
        rec16 = pools['attn'].tile([8, 512], BF16, tag="rec16")
        nc.vector.tensor_copy(out=rec16, in_=recf)
        for pt in range(4):
            bps = pools['bcast_psum'].tile([128, 512], F32, tag="sp")
            nc.tensor.matmul(bps, self.selbc[:, pt, :], rec16,
                             start=True, stop=True)
            # rows 0..104 only: row 105 of plane 3 is the constant bias-ones
            # row; pad rows 41..63 are zero so the multiply keeps them zero
            nc.vector.tensor_tensor(out=chunkbuf[:105, pt, cs],
                                    in0=chunkbuf[:105, pt, cs],
                                    in1=bps[:105, :], op=ALU.mult)

    def wo_residual(self, chunkbuf, w_o, c0, n512, pools):
        """xT[:, :, cols] += Wo_pad^T @ chunkbuf-slice (+bo via the constant
        ones row at plane 3 row 105)."""
        nc = self.nc
        cs = slice(n512 * 512, (n512 + 1) * 512)
        xcols = slice(c0 + n512 * 512, c0 + (n512 + 1) * 512)
        for mt in range(3):
            mrows = 128 if mt < 2 else 64
            ps = pools['psum'].tile([128, 512], F32, tag=f"proj{mt % 2}")
            for k in range(4):
                nc.tensor.matmul(ps[:mrows, :],
                                 w_o[:, k, mt * 128: mt * 128 + mrows],
                                 chunkbuf[:, k, cs],
                                 start=(k == 0), stop=(k == 3))
            nc.vector.tensor_tensor(out=self.xT[:mrows, mt, xcols],
                                    in0=ps[:mrows, :],
                                    in1=self.xT[:mrows, mt, xcols], op=ALU.add)

    def _mk_pools(self, sctx, extra=()):
        from contextlib import ExitStack
        tc = self.tc
        pools = {
            'ln': sctx.enter_context(tc.tile_pool(name="lnp", bufs=2)),
            'nhat': sctx.enter_context(tc.tile_pool(name="nhatp", bufs=3)),
            'psum': sctx.enter_context(tc.tile_pool(name="projps", bufs=1, space="PSUM")),
            'stat_psum': sctx.enter_context(tc.tile_pool(name="statps", bufs=1, space="PSUM")),
            'bcast_psum': sctx.enter_context(tc.tile_pool(name="bcps", bufs=1, space="PSUM")),
            'attn': sctx.enter_context(tc.tile_pool(name="attnp", bufs=2)),
        }
        return pools

    # ---------------- stage A: window attention ----------------
    def stage_window(self, sctx, w_q, w_k, w_v, w_o):
        nc, tc = self.nc, self.tc
        pools = self._mk_pools(sctx)
        qkvp = sctx.enter_context(tc.tile_pool(name="qkvA", bufs=1))
        spp = sctx.enter_context(tc.tile_pool(name="spA", bufs=2, space="PSUM"))
        avpp = sctx.enter_context(tc.tile_pool(name="avpA", bufs=2, space="PSUM"))
        epp = sctx.enter_context(tc.tile_pool(name="epA", bufs=2))

        for c0 in range(0, self.ntok, self.chunk):
            qT = qkvp.tile([128, 4, self.chunk], BF16, tag="qT")
            kT = qkvp.tile([128, 4, self.chunk], BF16, tag="kT")
            vP = qkvp.tile([128, self.chunk // 128, 328], BF16, tag="vP")
            self.qkv_chunk(c0, pools, w_q, w_k, w_v, qT, kT, vP)
            for wpair in range(self.chunk // 512):
                chunkbuf = pools['attn'].tile([128, 4, 512], BF16, tag="chunkbuf")
                nc.vector.memset(chunkbuf, 0.0)
                nc.sync.dma_start(out=chunkbuf[105:106, 3, :],
                                  in_=self.ones_row_bf[:1, :512])
                srows8 = pools['attn'].tile([8, 512], F32, tag="srows8")
                for wi in range(2):
                    t0 = wpair * 512 + wi * SEQ_TOK
                    ep = epp.tile([128, 2, SEQ_TOK], BF16, tag="ep")
                    for h in range(HEADS):
                        pt, r0 = hrow(h)
                        sp = spp.tile([128, 2, SEQ_TOK], F32, tag="sp")
                        for mt in range(2):
                            nc.tensor.matmul(
                                sp[:, mt, :],
                                kT[r0:r0 + DH, pt, t0 + mt * 128: t0 + (mt + 1) * 128],
                                qT[r0:r0 + DH, pt, t0: t0 + SEQ_TOK],
                                start=True, stop=True)
                        nc.scalar.activation(ep, sp, AF.Exp, scale=SCALE)
                        avp = avpp.tile([48, SEQ_TOK], F32, tag="avp")
                        for mt in range(2):
                            nc.tensor.matmul(
                                avp[:VDIM, :],
                                vP[:, (t0 // 128) + mt, h * VDIM: (h + 1) * VDIM],
                                ep[:, mt, :],
                                start=(mt == 0), stop=(mt == 1))
                        ccols = slice(wi * SEQ_TOK, (wi + 1) * SEQ_TOK)
                        nc.scalar.activation(chunkbuf[r0:r0 + VDIM, pt, ccols],
                                             avp[:VDIM, :], AF.Copy)
                        # arbitrary partition starts are DMA-only; DMA can't
                        # read PSUM, so take the sum row from chunkbuf (SBUF).
                        # gpsimd-initiated DMAs can cast bf16 -> f32.
                        nc.gpsimd.dma_start(out=srows8[h:h + 1, ccols],
                                            in_=chunkbuf[r0 + 32:r0 + 33, pt, ccols])
                self.finish_heads(chunkbuf, srows8, pools, 0)
                self.wo_residual(chunkbuf, w_o, c0 + wpair * 512, 0, pools)

    # ---------------- stage B/D: temporal attention ----------------
    def stage_temporal(self, sctx, w_q, w_k, w_v, w_o, tabq, tvrep):
        nc, tc = self.nc, self.tc
        from contextlib import ExitStack
        pools = self._mk_pools(sctx)
        qkvp = sctx.enter_context(tc.tile_pool(name="qkvT", bufs=1))
        spp = sctx.enter_context(tc.tile_pool(name="spT", bufs=2, space="PSUM"))
        avpp = sctx.enter_context(tc.tile_pool(name="avpT", bufs=2, space="PSUM"))
        rvpp = avpp
        epp = sctx.enter_context(tc.tile_pool(name="epT", bufs=2))

        nseq_c = self.chunk // T_LEN          # sequences per chunk
        ngrp_c = self.chunk // 128            # 8-seq groups per chunk
        # one chunkbuf for the whole stage: pad rows are zeroed once and the
        # finish multiply rewrites them as 0*recip = 0, so they stay zero
        chunkbuf = pools['attn'].tile([128, 4, self.chunk], BF16,
                                      tag="chunkbufT", bufs=1)
        nc.gpsimd.memset(chunkbuf, 0.0)
        nc.sync.dma_start(out=chunkbuf[105:106, 3, :],
                          in_=self.ones_row_bf[:1, :self.chunk])
        srows8 = pools['attn'].tile([8, self.chunk], F32,
                                    tag="srowsT", bufs=1)
        for c0 in range(0, self.ntok, self.chunk):
            qT = qkvp.tile([128, 4, self.chunk], BF16, tag="qT")
            kT = qkvp.tile([128, 4, self.chunk], BF16, tag="kT")
            vP = qkvp.tile([128, self.chunk // 128, 328], BF16, tag="vP")
            self.qkv_chunk(c0, pools, w_q, w_k, w_v, qT, kT, vP)

            # qaug[plane h//4, (h%4)*32+J, i*nseq_c + seq]
            #   = q_h[:, tok(seq,i)] . tabQ[:, i*16+J]
            qaug = qkvp.tile([128, 2, T_LEN * nseq_c], BF16, tag="qaug")
            i_per = 512 // nseq_c
            for plane in range(2):
                for r in range(T_LEN // i_per):
                    ps = spp.tile([128, 512], F32, tag="sp")
                    for ii in range(i_per):
                        i = r * i_per + ii
                        for hh in range(4):
                            h = plane * 4 + hh
                            pt, r0 = hrow(h)
                            nc.tensor.matmul(
                                ps[hh * 32: hh * 32 + 16,
                                   ii * nseq_c:(ii + 1) * nseq_c],
                                tabq[r0:r0 + DH, i * 16:(i + 1) * 16],
                                qT[r0:r0 + DH, pt, i::T_LEN],
                                start=True, stop=True,
                                tile_position=(r0, hh * 32))
                    for hh in range(4):
                        nc.scalar.activation(
                            qaug[hh * 32: hh * 32 + 16, plane,
                                 r * 512:(r + 1) * 512],
                            ps[hh * 32: hh * 32 + 16, :], AF.Copy)

            for h in range(HEADS):
                pt, r0 = hrow(h)
                qb = (h % 4) * 32      # qaug row base
                plane = h // 4
                ep = epp.tile([128, self.chunk], BF16, tag="ep")
                # scores in 4-group (512-col) batches: one mask add + one
                # exp per batch instead of per 128-col group
                for quad in range(ngrp_c // 4):
                    sp = spp.tile([128, 512], F32, tag="sp")
                    for g4 in range(4):
                        g = quad * 4 + g4
                        t0 = g * 128
                        qs = slice(g4 * 128, (g4 + 1) * 128)
                        nc.tensor.matmul(sp[:, qs],
                                         kT[r0:r0 + DH, pt, t0:t0 + 128],
                                         qT[r0:r0 + DH, pt, t0:t0 + 128],
                                         start=True, stop=False)
                        rhs = mkap(qaug, qb * qaug.ap[0][0]
                                   + plane * qaug.ap[1][0] + g * 8,
                                   [[qaug.ap[0][0], 16], [1, 8], [nseq_c, 16]])
                        nc.tensor.matmul(sp[:, qs], self.kaug[qb:qb + 16, :], rhs,
                                         start=False, stop=True,
                                         tile_position=(qb, 0))
                    nc.vector.tensor_tensor(out=sp, in0=sp, in1=self.mask4,
                                            op=ALU.add)
                    nc.scalar.activation(ep[:, quad * 512:(quad + 1) * 512],
                                         sp, AF.Exp, scale=SCALE)
                for quad in range(ngrp_c // 4):
                    avp = avpp.tile([128, 512], F32, tag="avp")
                    for g4 in range(4):
                        g = quad * 4 + g4
                        t0 = g * 128
                        nc.tensor.matmul(avp[r0:r0 + VDIM,
                                             g4 * 128:(g4 + 1) * 128],
                                         vP[:, g, h * VDIM: (h + 1) * VDIM],
                                         ep[:, t0:t0 + 128],
                                         start=True, stop=True,
                                         tile_position=(0, r0))
                    nc.scalar.activation(
                        chunkbuf[r0:r0 + VDIM, pt,
                                 quad * 512:(quad + 1) * 512],
                        avp[r0:r0 + VDIM, :], AF.Copy)
                # rel-v (writes 41 rows; the sum slot col of tvrep is zero)
                for rr in range(T_LEN // i_per):
                    rvp = rvpp.tile([128, 512], F32, tag="avp")
                    for ii in range(i_per):
                        i = rr * i_per + ii
                        nc.tensor.matmul(rvp[r0:r0 + VDIM,
                                             ii * nseq_c:(ii + 1) * nseq_c],
                                         tvrep[:, i * VDIM:(i + 1) * VDIM],
                                         ep[:, i::T_LEN], start=True, stop=True,
                                         tile_position=(0, r0))
                    dst = mkap(chunkbuf, r0 * chunkbuf.ap[0][0]
                               + pt * chunkbuf.ap[1][0] + rr * i_per,
                               [[chunkbuf.ap[0][0], VDIM], [T_LEN, nseq_c],
                                [1, i_per]])
                    src_ = mkap(rvp, r0 * rvp.ap[0][0],
                                [[rvp.ap[0][0], VDIM], [1, nseq_c],
                                 [nseq_c, i_per]])
                    nc.vector.tensor_tensor(out=dst, in0=dst, in1=src_, op=ALU.add)
                nc.gpsimd.dma_start(out=srows8[h:h + 1, :],
                                    in_=chunkbuf[r0 + 32:r0 + 33, pt, :])
            for n512 in range(self.chunk // 512):
                self.finish_heads(chunkbuf, srows8, pools, n512)
                self.wo_residual(chunkbuf, w_o, c0, n512, pools)

    # ---------------- stage C: cross attention ----------------
    def stage_cross(self, sctx, w_q, w_k, w_v, w_o):
        nc, tc = self.nc, self.tc
        pools = self._mk_pools(sctx)
        qkvp = sctx.enter_context(tc.tile_pool(name="qkvC", bufs=2))
        kvp = sctx.enter_context(tc.tile_pool(name="kvC", bufs=1))
        spp = sctx.enter_context(tc.tile_pool(name="spC", bufs=2, space="PSUM"))
        avpp = sctx.enter_context(tc.tile_pool(name="avpC", bufs=2, space="PSUM"))
        epp = sctx.enter_context(tc.tile_pool(name="epC", bufs=2))

        # K/V from context (once)
        ctxB = kvp.tile([128, 6, NCTX], BF16, name="ctxB")
        nc.vector.tensor_copy(out=ctxB, in_=self.ctxT)
        kT2 = kvp.tile([128, 4, NCTX], BF16, name="kT2")
        vP2 = kvp.tile([128, 328], BF16, name="vP2")
        for mt in range(4):
            ps = pools['psum'].tile([128, 512], F32, tag="proj0")
            for k in range(6):
                nc.tensor.matmul(ps[:, :NCTX],
                                 w_k[:, k, mt * 128: mt * 128 + 128],
                                 ctxB[:, k, :],
                                 start=(k == 0), stop=(k == 5))
            nc.vector.tensor_copy(out=kT2[:, mt, :], in_=ps[:, :NCTX])
        ps = pools['psum'].tile([128, 512], F32, tag="proj1")
        for k in range(6):
            nc.tensor.matmul(ps[:NCTX, :328], ctxB[:, k, :], w_v[:, k, :328],
                             start=(k == 0), stop=(k == 5))
        nc.vector.tensor_copy(out=vP2[:NCTX, :], in_=ps[:NCTX, :328])
        onescols = mkap(vP2, 32, [[vP2.ap[0][0], NCTX], [VDIM, 8]])
        nc.gpsimd.memset(onescols, 1.0)

        for c0 in range(0, self.ntok, self.chunk):
            qT = qkvp.tile([128, 4, self.chunk], BF16, tag="qT")
            for n in range(self.chunk // 512):
                tok0 = c0 + n * 512
                nhat = self.ln_stage(tok0, 512, pools)
                for mt in range(4):
                    ps = pools['psum'].tile([128, 512], F32, tag=f"proj{mt % 2}")
                    for ki, (k, rows) in enumerate(self.KS_AUG):
                        nc.tensor.matmul(ps,
                                         w_q[:rows, k, mt * 128: mt * 128 + 128],
                                         nhat[:rows, k, :],
                                         start=(ki == 0), stop=(ki == 2))
                    nc.scalar.activation(qT[:, mt, n * 512:(n + 1) * 512],
                                         ps, AF.Copy)
            for n in range(self.chunk // 512):
                ns = slice(n * 512, (n + 1) * 512)
                chunkbuf = pools['attn'].tile([128, 4, 512], BF16, tag="chunkbuf")
                nc.vector.memset(chunkbuf, 0.0)
                nc.sync.dma_start(out=chunkbuf[105:106, 3, :],
                                  in_=self.ones_row_bf[:1, :512])
                srows8 = pools['attn'].tile([8, 512], F32, tag="srows8")
                for h in range(HEADS):
                    pt, r0 = hrow(h)
                    sp = spp.tile([128, 512], F32, tag="sp")
                    nc.tensor.matmul(sp[:NCTX, :], kT2[r0:r0 + DH, pt, :],
                                     qT[r0:r0 + DH, pt, ns],
                                     start=True, stop=True)
                    ep = epp.tile([128, 512], BF16, tag="ep")
                    nc.scalar.activation(ep[:NCTX, :], sp[:NCTX, :], AF.Exp, scale=SCALE)
                    avp = avpp.tile([48, 512], F32, tag="avp")
                    nc.tensor.matmul(avp[:VDIM, :],
                                     vP2[:NCTX, h * VDIM: (h + 1) * VDIM],
                                     ep[:NCTX, :], start=True, stop=True)
                    nc.scalar.activation(chunkbuf[r0:r0 + VDIM, pt, :],
                                         avp[:VDIM, :], AF.Copy)
                    nc.gpsimd.dma_start(out=srows8[h:h + 1, :],
                                        in_=chunkbuf[r0 + 32:r0 + 33, pt, :])
                self.finish_heads(chunkbuf, srows8, pools, 0)
                self.wo_residual(chunkbuf, w_o, c0 + n * 512, 0, pools)

    # ---------------- stage E: GEGLU FF ----------------
    def stage_ff(self, sctx, w1, w2):
        nc, tc = self.nc, self.tc
        pools = self._mk_pools(sctx)
        ffp = sctx.enter_context(tc.tile_pool(name="ffp", bufs=2))
        gpsum = sctx.enter_context(tc.tile_pool(name="gps", bufs=2, space="PSUM"))

        for n in range(self.ntok // 512):
            tok0 = n * 512
            nhat = self.ln_stage(tok0, 512, pools)
            ff = ffp.tile([128, 11, 512], BF16, tag="ff")
            for mt in range(10):
                aps = gpsum.tile([128, 512], F32, tag="apsum")
                gps = gpsum.tile([128, 512], F32, tag="gpsum")
                for ki, (k, rows) in enumerate(self.KS_AUG):
                    nc.tensor.matmul(aps,
                                     w1[:rows, k, mt * 128: mt * 128 + 128],
                                     nhat[:rows, k, :],
                                     start=(ki == 0), stop=(ki == 2))
                    nc.tensor.matmul(gps,
                                     w1[:rows, k, FF + mt * 128: FF + mt * 128 + 128],
                                     nhat[:rows, k, :],
                                     start=(ki == 0), stop=(ki == 2))
                gelu = pools['ln'].tile([128, 512], BF16, tag="gelu")
                if self.sim_gelu:
                    # CoreSim lacks Gelu: x*sigmoid(1.702x) stand-in, matched
                    # by the hostref flag. HW uses the real erf Gelu below.
                    nc.scalar.activation(gelu, gps, AF.Sigmoid, scale=1.702)
                    nc.vector.tensor_tensor(out=gelu, in0=gps, in1=gelu,
                                            op=ALU.mult)
                else:
                    nc.scalar.activation(gelu, gps, AF.Gelu)
                nc.vector.tensor_tensor(out=ff[:, mt, :], in0=aps, in1=gelu,
                                        op=ALU.mult)
            nc.gpsimd.memset(ff[0:1, 10, :], 1.0)
            for mt in range(3):
                mrows = 128 if mt < 2 else 64
                ps = pools['psum'].tile([128, 512], F32, tag=f"proj{mt % 2}")
                for k in range(10):
                    nc.tensor.matmul(ps[:mrows, :],
                                     w2[:, k, mt * 128: mt * 128 + mrows],
                                     ff[:, k, :], start=(k == 0), stop=False)
                nc.tensor.matmul(ps[:mrows, :],
                                 w2[0:1, 10, mt * 128: mt * 128 + mrows],
                                 ff[0:1, 10, :], start=False, stop=True)
                cols = slice(tok0, tok0 + 512)
                nc.vector.tensor_tensor(out=self.xT[:mrows, mt, cols],
                                        in0=ps[:mrows, :],
                                        in1=self.xT[:mrows, mt, cols], op=ALU.add)


# ----------------------------------------------------------------------------
# host entry point
# ----------------------------------------------------------------------------

_nc_cache = {}


def _get_nc(nwin=16, chunk_win=8, stages=('a1', 't1', 'a2', 't2', 'ff')):
    key = (nwin, chunk_win, stages)
    if key not in _nc_cache:
        _nc_cache[key] = Builder(nwin, chunk_win, stages).build(
            num_devices=NCORES)
    return _nc_cache[key]


def make_in_maps(inputs, nwin=16):
    x = np.asarray(inputs['x'], np.float32)
    context = np.asarray(inputs['context'], np.float32)
    wd = {k: np.asarray(v, np.float32) for k, v in inputs.items()
          if k not in ('x', 'context')}
    wt = prep_weights(wd)
    shards = shard_x(x, nwin)
    ncore = shards.shape[0]
    in_maps = []
    for c in range(ncore):
        bidx = (c * nwin) // (NH * NH)
        ctxT = _cmajor(np.ascontiguousarray(context[bidx].T), 768)  # [128,6,77]
        m = {'xT': np.ascontiguousarray(_cmajor(shards[c], 384), dtype=np.float32),
             'ctxT': np.ascontiguousarray(ctxT, dtype=np.float32)}
        m.update(wt)
        in_maps.append(m)
    return in_maps


def kernel(**inputs):
    nwin = 16
    nc = _get_nc(nwin)
    in_maps = make_in_maps(inputs, nwin)
    res = run_bass_kernel_spmd(nc, in_maps, list(range(NCORES)))
    outs = np.stack([r['yT'] for r in res.results])  # [8, 128, 3, ntok]
    # undo c-major padding: [8, 128, 3, ntok] -> [8, 384, ntok] -> [8, 320, ntok]
    outs = outs.transpose(0, 2, 1, 3).reshape(NCORES, 384, nwin * SEQ_TOK)[:, :D]
    return unshard_x(outs, nwin).astype(np.float32)



# revision 32
# speedup vs baseline: 1.2175x; 1.2175x over previous
# Trainium2 Bass kernel for nn_BasicTransformerBlockST (spatio-temporal
# transformer block: windowed spatial self-attention, two temporal
# self-attentions with relative-position bias + causal mask, cross-attention
# to a text context, and a GEGLU feed-forward).
#
# Sharding: data-parallel over the 128 (b, nh, nw) spatial windows -> 16
# windows x 4096 tokens per core; every stage (window attn / temporal attn /
# cross attn / FF) is closed under this shard, so no collectives are needed.
#
# Per-core layout: activations channel-major xT [C=320, ntok] resident in SBUF
# as [128, 3, ntok]; token order (window, spatial, t) makes temporal sequences
# contiguous 16-token runs. Softmax runs in transposed score space S^T[k, q]
# without max subtraction (logits are O(1)); the normalizer comes from an
# appended ones-column in V and is applied post-attention. Masked entries get
# -1e5 before exp and underflow to exactly 0, which makes the block-diagonal
# batched temporal attention exact. LayerNorm is folded: gamma/beta fold into
# the projection weights; the -mu*rstd and beta terms ride along as augmented
# contraction rows.
import numpy as np
import ml_dtypes

import concourse.bass as bass
import concourse.tile as tile
from concourse import bacc, mybir
from concourse.bass_utils import run_bass_kernel_spmd

F32 = mybir.dt.float32
F32R = mybir.dt.float32r
BF16 = mybir.dt.bfloat16
AF = mybir.ActivationFunctionType
ALU = mybir.AluOpType

D, CTX_DIM, HEADS, DH, T_LEN, WS, MAXREL, FF = 320, 768, 8, 40, 16, 4, 16, 1280
B, H, W = 2, 32, 32
NH = H // WS
NWIN = B * NH * NH          # 128 windows total
NCORES = 8
SEQ_TOK = T_LEN * WS * WS   # 256 tokens per window
SCALE = DH ** -0.5
NEG = -1e5
NCTX = 77
EPS = 1e-5

bfdt = ml_dtypes.bfloat16


# ----------------------------------------------------------------------------
# host-side data prep
# ----------------------------------------------------------------------------

def shard_x(x, win_per_core):
    xr = np.asarray(x, np.float32).reshape(B, D, T_LEN, NH, WS, NH, WS)
    xr = xr.transpose(0, 3, 5, 1, 4, 6, 2)          # B nh nw C wh ww T
    xr = xr.reshape(NWIN, D, WS * WS * T_LEN)       # win C (s t)
    ncore = NWIN // win_per_core
    xr = xr.reshape(ncore, win_per_core, D, WS * WS * T_LEN)
    xr = xr.transpose(0, 2, 1, 3).reshape(ncore, D, win_per_core * WS * WS * T_LEN)
    return np.ascontiguousarray(xr)


def unshard_x(shards, win_per_core):
    ncore = NWIN // win_per_core
    xr = shards.reshape(ncore, D, win_per_core, WS * WS * T_LEN).transpose(0, 2, 1, 3)
    xr = xr.reshape(B, NH, NH, D, WS, WS, T_LEN)
    xr = xr.transpose(0, 3, 6, 1, 4, 2, 5)          # B C T nh wh nw ww
    return np.ascontiguousarray(xr.reshape(B, D, T_LEN, H, W))


def _cmajor(a, rows):
    """[rows_logical<=rows, cols] -> [128, rows/128, cols], zero padded."""
    out = np.zeros((rows, a.shape[1]), np.float32)
    out[: a.shape[0]] = a
    return np.ascontiguousarray(
        out.reshape(rows // 128, 128, a.shape[1]).transpose(1, 0, 2))


VDIM = DH + 1    # 41 per-head value columns; slot 32 is the ones column


def vslot(c):
    """map v-slot index c in [0,41) to head dim, or None for the ones slot."""
    if c == 32:
        return None
    return c if c < 32 else c - 1


def pad_v_cols(Wv):
    """[cin, 320] -> [cin, 328]: per-head 41 columns; slot 32 left zero
    (filled with ones on device for the softmax-denominator trick)."""
    cin = Wv.shape[0]
    out = np.zeros((cin, HEADS * VDIM), np.float32)
    for h in range(HEADS):
        for c in range(VDIM):
            d = vslot(c)
            if d is not None:
                out[:, h * VDIM + c] = Wv[:, h * DH + d]
    return out


def pad_head_cols(Wx):
    """[cin, 320] -> [cin, 512]: head h cols at h*64+[0,40), zeros between."""
    out = np.zeros((Wx.shape[0], 512), np.float32)
    for h in range(HEADS):
        out[:, h * 64: h * 64 + 40] = Wx[:, h * 40: (h + 1) * 40]
    return out


def prep_proj_w(Wraw, gamma, beta, extra_bias=None, pad_heads=False):
    """Augmented c-major projection weight [128, 3, dout]:
    rows 0..320 = W*gamma[:,None]; row (2,64) = colsum (pairs with -mu*rstd);
    row (2,65) = beta@Wg (+extra_bias)."""
    Wg = np.asarray(Wraw, np.float32) * np.asarray(gamma, np.float32)[:, None]
    if pad_heads:
        Wg = pad_head_cols(Wg)
    out = np.zeros((384, Wg.shape[1]), np.float32)
    out[:320] = Wg
    out[256 + 64] = Wg.sum(0)
    out[256 + 65] = np.asarray(beta, np.float32) @ Wg
    if extra_bias is not None:
        out[256 + 65] += np.asarray(extra_bias, np.float32)
    return _cmajor(out, 384)


def prep_wo(Wo, bo):
    """[320, 320] -> lhsT [128, 4, 320]: head h rows at h*64+c for v-slot c
    (zero at the sum slot c=32); bias rides at plane-3 row 105 (a pad row
    that the device keeps at constant 1.0)."""
    out = np.zeros((512, 320), np.float32)
    Wo = np.asarray(Wo, np.float32)
    for h in range(HEADS):
        for c in range(VDIM):
            d = vslot(c)
            if d is not None:
                out[h * 64 + c] = Wo[h * DH + d]
    out[3 * 128 + 105] = np.asarray(bo, np.float32)
    return _cmajor(out, 512)


def prep_tabq(table):
    """relk [33, 40] -> tabQ [128, 256]: col (i*16+J) holds table[J-i+16] in
    rows 0..40 AND a copy in rows 64..104 (so lhsT base matches q's base)."""
    out = np.zeros((128, 256), np.float32)
    t = np.asarray(table, np.float32)
    for i in range(T_LEN):
        for J in range(T_LEN):
            out[:40, i * 16 + J] = t[J - i + MAXREL]
            out[64:104, i * 16 + J] = t[J - i + MAXREL]
    return out


def prep_tvrep(table):
    """relv [33, 40] -> tvrep [128, 16*41]: for query pos i, col i*41+c
    (v-slot c; zero at c=32) row (s*16+j) holds table[j-i+16, dim(c)]."""
    out = np.zeros((128, T_LEN * VDIM), np.float32)
    t = np.asarray(table, np.float32)
    for i in range(T_LEN):
        for s in range(8):
            for j in range(T_LEN):
                for c in range(VDIM):
                    d = vslot(c)
                    if d is not None:
                        out[s * 16 + j, i * VDIM + c] = t[j - i + MAXREL, d]
    return out


def prep_kaug():
    """constant selector [128, 128]: rows b+J (for each base b in
    0/32/64/96) one at cols (s*16+J)."""
    out = np.zeros((128, 128), np.float32)
    for base in (0, 32, 64, 96):
        for s in range(8):
            for J in range(T_LEN):
                out[base + J, s * 16 + J] = 1.0
    return out


def prep_mask():
    """additive [128, 128]: row (s,j), col (s',i): 0 iff s==s' and j<=i."""
    m = np.full((128, 128), NEG, np.float32)
    for s in range(8):
        for j in range(T_LEN):
            m[s * 16 + j, s * 16 + j: (s + 1) * 16] = 0.0
    return m


def prep_selbc():
    """[8, 4, 128]: row h, plane pt: ones over head h's 64-row block."""
    out = np.zeros((8, 4, 128), np.float32)
    for h in range(8):
        out[h, h // 2, 64 * (h % 2): 64 * (h % 2) + 64] = 1.0
    return out


def prep_weights(wd):
    t = {}

    def bfc(x):
        return np.ascontiguousarray(np.asarray(x, np.float32).astype(bfdt))

    for nm, g, b in (('a1', wd['ln1_g'], wd['ln1_b']),
                     ('t1', wd['ln4_g'], wd['ln4_b']),
                     ('t2', wd['ln5_g'], wd['ln5_b'])):
        t[f'{nm}_wq'] = bfc(prep_proj_w(wd[f'{nm}_Wq'], g, b, pad_heads=True))
        t[f'{nm}_wk'] = bfc(prep_proj_w(wd[f'{nm}_Wk'], g, b, pad_heads=True))
        Wvp = pad_v_cols(np.asarray(wd[f'{nm}_Wv'], np.float32))
        t[f'{nm}_wv'] = bfc(prep_proj_w(Wvp, g, b))
        t[f'{nm}_wo'] = bfc(prep_wo(wd[f'{nm}_Wo'], wd[f'{nm}_bo']))
    t['a2_wq'] = bfc(prep_proj_w(wd['a2_Wq'], wd['ln2_g'], wd['ln2_b'],
                                 pad_heads=True))
    t['a2_wk'] = bfc(_cmajor(pad_head_cols(np.asarray(wd['a2_Wk'], np.float32)), 768))
    t['a2_wv'] = bfc(_cmajor(pad_v_cols(np.asarray(wd['a2_Wv'], np.float32)), 768))
    t['a2_wo'] = bfc(prep_wo(wd['a2_Wo'], wd['a2_bo']))
    t['ff_w1'] = bfc(prep_proj_w(wd['ff_W1'], wd['ln3_g'], wd['ln3_b'],
                                 extra_bias=wd['ff_b1']))
    W2aug = np.zeros((1408, 320), np.float32)
    W2aug[:1280] = np.asarray(wd['ff_W2'], np.float32)
    W2aug[1280] = np.asarray(wd['ff_b2'], np.float32)
    t['ff_w2'] = bfc(_cmajor(W2aug, 1408))
    t['t1_tabq'] = bfc(prep_tabq(wd['t1_relk']))
    t['t2_tabq'] = bfc(prep_tabq(wd['t2_relk']))
    t['t1_tvrep'] = bfc(prep_tvrep(wd['t1_relv']))
    t['t2_tvrep'] = bfc(prep_tvrep(wd['t2_relv']))
    t['kaug'] = bfc(prep_kaug())
    t['mask4'] = np.ascontiguousarray(np.tile(prep_mask(), (1, 4)))
    t['selbc'] = bfc(prep_selbc())
    return t


WEIGHT_SHAPES = {}
for _s in ('a1', 't1', 't2'):
    WEIGHT_SHAPES.update({f'{_s}_wq': ([128, 3, 512], BF16),
                          f'{_s}_wk': ([128, 3, 512], BF16),
                          f'{_s}_wv': ([128, 3, 328], BF16),
                          f'{_s}_wo': ([128, 4, 320], BF16)})
WEIGHT_SHAPES.update({
    'a2_wq': ([128, 3, 512], BF16), 'a2_wk': ([128, 6, 512], BF16),
    'a2_wv': ([128, 6, 328], BF16), 'a2_wo': ([128, 4, 320], BF16),
    'ff_w1': ([128, 3, 2560], BF16), 'ff_w2': ([128, 11, 320], BF16),
    't1_tabq': ([128, 256], BF16), 't2_tabq': ([128, 256], BF16),
    't1_tvrep': ([128, 656], BF16), 't2_tvrep': ([128, 656], BF16),
    'kaug': ([128, 128], BF16), 'mask4': ([128, 512], F32),
    'selbc': ([8, 4, 128], BF16),
})

STAGE_WEIGHTS = {
    'a1': ['a1_wq', 'a1_wk', 'a1_wv', 'a1_wo'],
    't1': ['t1_wq', 't1_wk', 't1_wv', 't1_wo', 't1_tabq', 't1_tvrep'],
    'a2': ['a2_wq', 'a2_wk', 'a2_wv', 'a2_wo'],
    't2': ['t2_wq', 't2_wk', 't2_wv', 't2_wo', 't2_tabq', 't2_tvrep'],
    'ff': ['ff_w1', 'ff_w2'],
}


# ----------------------------------------------------------------------------
# device kernel builder
# ----------------------------------------------------------------------------

def hrow(h):
    """(ptile, row0) of head h in the head-padded 512-row q/k layout."""
    return h // 2, (h % 2) * 64


def mkap(t, extra_off, dims):
    return bass.AP(tensor=t.tensor, offset=t.offset + extra_off, ap=[list(d) for d in dims])


class Builder:
    def __init__(self, nwin=16, chunk_win=4, stages=('a1', 't1', 'a2', 't2', 'ff'),
                 sim_gelu=False):
        self.sim_gelu = sim_gelu
        self.nwin = nwin
        self.ntok = nwin * SEQ_TOK
        self.chunk = min(chunk_win * SEQ_TOK, self.ntok)   # tokens per chunk
        self.stages = stages

    def build(self, num_devices=1):
        nc = bacc.Bacc("TRN2", target_bir_lowering=False, debug=False,
                       num_devices=num_devices)
        self.nc = nc
        dram = {}
        dram['xT'] = nc.declare_dram_parameter('xT', [128, 3, self.ntok], F32,
                                               isOutput=False)
        dram['ctxT'] = nc.declare_dram_parameter('ctxT', [128, 6, NCTX], F32,
                                                 isOutput=False)
        for nm, (shp, dt) in WEIGHT_SHAPES.items():
            dram[nm] = nc.declare_dram_parameter(nm, list(shp), dt, isOutput=False)
        out_yT = nc.declare_dram_parameter('yT', [128, 3, self.ntok], F32,
                                           isOutput=True)
        if getattr(self, 'debug_taps', False):
            dram['dbg_q'] = nc.declare_dram_parameter(
                'dbg_q', [128, 4, self.chunk], BF16, isOutput=True)
            dram['dbg_n'] = nc.declare_dram_parameter(
                'dbg_n', [128, 3, 512], BF16, isOutput=True)
            dram['dbg_cb'] = nc.declare_dram_parameter(
                'dbg_cb', [128, 5, 512], BF16, isOutput=True)
            dram['dbg_s'] = nc.declare_dram_parameter(
                'dbg_s', [128, 4, 512], F32, isOutput=True)
        self.dram = dram
        with tile.TileContext(nc) as tc:
            self.tc = tc
            self._emit(out_yT)
        nc.compile()
        return nc

    # ---------------- helpers ----------------
    def _emit(self, out_yT):
        from contextlib import ExitStack
        nc, tc = self.nc, self.tc
        with ExitStack() as ctx:
            resid = ctx.enter_context(tc.tile_pool(name="resid", bufs=1))
            consts = ctx.enter_context(tc.tile_pool(name="consts", bufs=1))

            xT = resid.tile([128, 3, self.ntok], F32)
            nc.sync.dma_start(out=xT, in_=self.dram['xT'][:])
            self.xT = xT

            self.ones_col = consts.tile([128, 1], F32, name="onescol")
            nc.vector.memset(self.ones_col, 1.0)
            self.ones_col_bf = consts.tile([128, 1], BF16, name="onescolbf")
            nc.vector.memset(self.ones_col_bf, 1.0)
            self.eps_col = consts.tile([128, 1], F32, name="epscol")
            nc.vector.memset(self.eps_col, EPS)
            self.ones_row_bf = consts.tile([1, 2048], BF16, name="onesrowbf")
            nc.vector.memset(self.ones_row_bf, 1.0)
            self.mask4 = consts.tile([128, 512], F32, name="mask4c")
            nc.sync.dma_start(out=self.mask4, in_=self.dram['mask4'][:])
            self.kaug = consts.tile([128, 128], BF16, name="kaugc")
            nc.sync.dma_start(out=self.kaug, in_=self.dram['kaug'][:])
            self.selbc = consts.tile([8, 4, 128], BF16, name="selbcc")
            nc.sync.dma_start(out=self.selbc, in_=self.dram['selbc'][:])
            # context stays resident (tiny)
            self.ctxT = consts.tile([128, 6, NCTX], F32, name="ctxTc")
            nc.sync.dma_start(out=self.ctxT, in_=self.dram['ctxT'][:])

            for st in self.stages:
                with ExitStack() as sctx:
                    wpool = sctx.enter_context(
                        tc.tile_pool(name=f"w_{st}", bufs=1))
                    w = {}
                    for nm in STAGE_WEIGHTS[st]:
                        shp, dt = WEIGHT_SHAPES[nm]
                        w[nm] = wpool.tile(list(shp), dt, name=f"sb_{nm}")
                        nc.sync.dma_start(out=w[nm], in_=self.dram[nm][:])
                    if st == 'a1':
                        self.stage_window(sctx, w['a1_wq'], w['a1_wk'],
                                          w['a1_wv'], w['a1_wo'])
                    elif st in ('t1', 't2'):
                        self.stage_temporal(sctx, w[f'{st}_wq'], w[f'{st}_wk'],
                                            w[f'{st}_wv'], w[f'{st}_wo'],
                                            w[f'{st}_tabq'], w[f'{st}_tvrep'])
                    elif st == 'a2':
                        self.stage_cross(sctx, w['a2_wq'], w['a2_wk'],
                                         w['a2_wv'], w['a2_wo'])
                    elif st == 'ff':
                        self.stage_ff(sctx, w['ff_w1'], w['ff_w2'])

            nc.sync.dma_start(out=out_yT[:], in_=xT)

    def ln_stage(self, tok0, ncols, pools):
        """nhat [128, 3, ncols] bf16: rows 0..320 = (x-mu)*rstd;
        row (2,64) = -mu*rstd; row (2,65) = 1.0."""
        nc = self.nc
        xT = self.xT
        stat_psum = pools['stat_psum'].tile([33, 512], F32, tag="statp")
        sq = pools['ln'].tile([128, 3, 512], BF16, tag="lnsq")
        xb = pools['ln'].tile([128, 3, 512], BF16, tag="lnxb")
        cols = slice(tok0, tok0 + ncols)
        # single 3-plane ops: plane-2 pad rows are zero in xT, so summing the
        # full 128 rows of every plane changes nothing
        nc.vector.tensor_copy(out=xb[:, :, :ncols], in_=xT[:, :, cols])
        nc.scalar.square(sq[:, :, :ncols], xT[:, :, cols])
        for k in range(3):
            nc.tensor.matmul(stat_psum[0:1, :ncols],
                             self.ones_col_bf[:128],
                             xb[:, k, :ncols],
                             start=(k == 0), stop=(k == 2))
            nc.tensor.matmul(stat_psum[32:33, :ncols],
                             self.ones_col_bf[:128],
                             sq[:, k, :ncols],
                             start=(k == 0), stop=(k == 2))
        # single-row stat tiles, all at partition 0 (walrus requires equal
        # SBUF start partitions within one TensorTensor op)
        st_mu = pools['ln'].tile([1, 512], F32, tag="lnmu")
        st_ex2 = pools['ln'].tile([1, 512], F32, tag="lnex2")
        st_rstd = pools['ln'].tile([1, 512], F32, tag="lnrstd")
        st_murstd = pools['ln'].tile([1, 512], F32, tag="lnmurstd")
        nc.vector.tensor_scalar_mul(st_mu[:, :ncols], stat_psum[0:1, :ncols], 1.0 / D)
        nc.vector.tensor_scalar_mul(st_ex2[:, :ncols], stat_psum[32:33, :ncols], 1.0 / D)
        nc.vector.tensor_tensor(out=st_rstd[:, :ncols], in0=st_mu[:, :ncols],
                                in1=st_mu[:, :ncols], op=ALU.mult)
        nc.vector.tensor_tensor(out=st_rstd[:, :ncols], in0=st_ex2[:, :ncols],
                                in1=st_rstd[:, :ncols], op=ALU.subtract)
        nc.scalar.activation(st_rstd[:, :ncols], st_rstd[:, :ncols], AF.Sqrt,
                             bias=self.eps_col[:1])
        st_rscr = pools['ln'].tile([1, 512], F32, tag="lnrscr")
        nc.vector.reciprocal_approx_accurate(st_rstd[:, :ncols],
                                             st_rstd[:, :ncols],
                                             st_rscr[:, :ncols])
        nc.vector.tensor_tensor(out=st_murstd[:, :ncols], in0=st_mu[:, :ncols],
                                in1=st_rstd[:, :ncols], op=ALU.mult)
        nc.vector.tensor_scalar_mul(st_murstd[:, :ncols], st_murstd[:, :ncols], -1.0)
        rstd_b = pools['ln'].tile([128, 512], F32, tag="lnrstdb")
        nc.gpsimd.partition_broadcast(rstd_b[:, :ncols], st_rstd[:, :ncols])
        nhat = pools['nhat'].tile([128, 3, 512], BF16, tag="nhat")
        # one 3-plane multiply; rstd_b repeats across planes via a 0-stride dim
        rstd3 = mkap(rstd_b, 0, [[rstd_b.ap[0][0], 128], [0, 3], [1, ncols]])
        nc.vector.tensor_tensor(out=nhat[:, :, :ncols],
                                in0=xT[:, :, cols],
                                in1=rstd3, op=ALU.mult)
        nhat_mr = pools['ln'].tile([1, 512], BF16, tag="lnmrbf")
        nc.vector.tensor_copy(out=nhat_mr[:, :ncols], in_=st_murstd[:, :ncols])
        # aug rows via DMA (arbitrary partition starts are DMA-only)
        nc.sync.dma_start(out=nhat[64:65, 2, :ncols], in_=nhat_mr[:1, :ncols])
        nc.sync.dma_start(out=nhat[65:66, 2, :ncols],
                          in_=self.ones_row_bf[:1, :ncols])
        return nhat

    KS_AUG = ((0, 128), (1, 128), (2, 66))
    KS_NOAUG = ((0, 128), (1, 128), (2, 64))

    def qkv_chunk(self, c0, pools, w_q, w_k, w_v, qT, kT, vP):
        """LN + q/k/v for tokens [c0, c0+chunk): qT,kT [128,4,chunk] bf16
        c-major head-padded; vP [128, chunk/128, 328] bf16 token-major."""
        nc = self.nc
        ntt = self.chunk // 512
        for n in range(ntt):
            tok0 = c0 + n * 512
            nhat = self.ln_stage(tok0, 512, pools)
            if getattr(self, 'debug_taps', False) and tok0 == 0:
                nc.sync.dma_start(out=self.dram['dbg_n'][:], in_=nhat)
            for wsb, dst, eng in ((w_q, qT, 'act'), (w_k, kT, 'dve')):
                for mt in range(4):
                    ps = pools['psum'].tile([128, 512], F32, tag=f"proj{mt % 2}")
                    for ki, (k, rows) in enumerate(self.KS_AUG):
                        nc.tensor.matmul(ps,
                                         wsb[:rows, k, mt * 128: mt * 128 + 128],
                                         nhat[:rows, k, :],
                                         start=(ki == 0), stop=(ki == 2))
                    dcols = slice(n * 512, (n + 1) * 512)
                    if eng == 'act':
                        nc.scalar.activation(dst[:, mt, dcols], ps, AF.Copy)
                    else:
                        nc.vector.tensor_copy(out=dst[:, mt, dcols], in_=ps)
            for m in range(4):
                ps = pools['psum'].tile([128, 512], F32, tag=f"proj{m % 2}")
                for ki, (k, rows) in enumerate(self.KS_AUG):
                    nc.tensor.matmul(ps[:, :328],
                                     nhat[:rows, k, m * 128:(m + 1) * 128],
                                     w_v[:rows, k, :328],
                                     start=(ki == 0), stop=(ki == 2))
                blk = n * 4 + m
                if m % 2 == 0:
                    nc.scalar.activation(vP[:, blk, :328], ps[:, :328], AF.Copy)
                else:
                    nc.vector.tensor_copy(out=vP[:, blk, :328], in_=ps[:, :328])
        # ones columns: vP[:, :, 32::41]
        onescols = mkap(vP, 32, [[vP.ap[0][0], 128],
                                 [328, self.chunk // 128], [41, 8]])
        nc.gpsimd.memset(onescols, 1.0)

    def finish_heads(self, chunkbuf, srows8, pools, n512):
        """normalize chunkbuf[:, :4, cs] by per-head recips.
        srows8 [8, >=512] f32: head h sum at row h. One reciprocal over the
        8 used rows, then a K=8 selector matmul broadcasts each head's recip
        across its 64-row block, one plane at a time."""
        nc = self.nc
        cs = slice(n512 * 512, (n512 + 1) * 512)
        recf = pools['attn'].tile([8, 512], F32, tag="recipf")
        nc.vector.reciprocal_approx_fast(recf, srows8[:, cs])
        rec16 = pools['attn'].tile([8, 512], BF16, tag="rec16")
        nc.vector.tensor_copy(out=rec16, in_=recf)
        for pt in range(4):
            bps = pools['bcast_psum'].tile([128, 512], F32, tag="sp")
            nc.tensor.matmul(bps, self.selbc[:, pt, :], rec16,
                             start=True, stop=True)
            # rows 0..104 only: row 105 of plane 3 is the constant bias-ones
            # row; pad rows 41..63 are zero so the multiply keeps them zero
            nc.vector.tensor_tensor(out=chunkbuf[:105, pt, cs],
                                    in0=chunkbuf[:105, pt, cs],
                                    in1=bps[:105, :], op=ALU.mult)

    def wo_residual(self, chunkbuf, w_o, c0, n512, pools):
        """xT[:, :, cols] += Wo_pad^T @ chunkbuf-slice (+bo via the constant
        ones row at plane 3 row 105)."""
        nc = self.nc
        cs = slice(n512 * 512, (n512 + 1) * 512)
        xcols = slice(c0 + n512 * 512, c0 + (n512 + 1) * 512)
        for mt in range(3):
            mrows = 128 if mt < 2 else 64
            ps = pools['psum'].tile([128, 512], F32, tag=f"proj{mt % 2}")
            for k in range(4):
                nc.tensor.matmul(ps[:mrows, :],
                                 w_o[:, k, mt * 128: mt * 128 + mrows],
                                 chunkbuf[:, k, cs],
                                 start=(k == 0), stop=(k == 3))
            nc.vector.tensor_tensor(out=self.xT[:mrows, mt, xcols],
                                    in0=ps[:mrows, :],
                                    in1=self.xT[:mrows, mt, xcols], op=ALU.add)

    def _mk_pools(self, sctx, extra=()):
        from contextlib import ExitStack
        tc = self.tc
        pools = {
            'ln': sctx.enter_context(tc.tile_pool(name="lnp", bufs=2)),
            'nhat': sctx.enter_context(tc.tile_pool(name="nhatp", bufs=3)),
            'psum': sctx.enter_context(tc.tile_pool(name="projps", bufs=1, space="PSUM")),
            'stat_psum': sctx.enter_context(tc.tile_pool(name="statps", bufs=1, space="PSUM")),
            'bcast_psum': sctx.enter_context(tc.tile_pool(name="bcps", bufs=1, space="PSUM")),
            'attn': sctx.enter_context(tc.tile_pool(name="attnp", bufs=2)),
        }
        return pools

    # ---------------- stage A: window attention ----------------
    def stage_window(self, sctx, w_q, w_k, w_v, w_o):
        nc, tc = self.nc, self.tc
        pools = self._mk_pools(sctx)
        qkvp = sctx.enter_context(tc.tile_pool(name="qkvA", bufs=1))
        spp = sctx.enter_context(tc.tile_pool(name="spA", bufs=2, space="PSUM"))
        avpp = sctx.enter_context(tc.tile_pool(name="avpA", bufs=2, space="PSUM"))
        epp = sctx.enter_context(tc.tile_pool(name="epA", bufs=2))

        for c0 in range(0, self.ntok, self.chunk):
            qT = qkvp.tile([128, 4, self.chunk], BF16, tag="qT")
            kT = qkvp.tile([128, 4, self.chunk], BF16, tag="kT")
            vP = qkvp.tile([128, self.chunk // 128, 328], BF16, tag="vP")
            self.qkv_chunk(c0, pools, w_q, w_k, w_v, qT, kT, vP)
            for wpair in range(self.chunk // 512):
                chunkbuf = pools['attn'].tile([128, 4, 512], BF16, tag="chunkbuf")
                nc.vector.memset(chunkbuf, 0.0)
                nc.sync.dma_start(out=chunkbuf[105:106, 3, :],
                                  in_=self.ones_row_bf[:1, :512])
                srows8 = pools['attn'].tile([8, 512], F32, tag="srows8")
                for wi in range(2):
                    t0 = wpair * 512 + wi * SEQ_TOK
                    ep = epp.tile([128, 2, SEQ_TOK], BF16, tag="ep")
                    for h in range(HEADS):
                        pt, r0 = hrow(h)
                        sp = spp.tile([128, 2, SEQ_TOK], F32, tag="sp")
                        for mt in range(2):
                            nc.tensor.matmul(
                                sp[:, mt, :],
                                kT[r0:r0 + DH, pt, t0 + mt * 128: t0 + (mt + 1) * 128],
                                qT[r0:r0 + DH, pt, t0: t0 + SEQ_TOK],
                                start=True, stop=True)
                        nc.scalar.activation(ep, sp, AF.Exp, scale=SCALE)
                        avp = avpp.tile([48, SEQ_TOK], F32, tag="avp")
                        for mt in range(2):
                            nc.tensor.matmul(
                                avp[:VDIM, :],
                                vP[:, (t0 // 128) + mt, h * VDIM: (h + 1) * VDIM],
                                ep[:, mt, :],
                                start=(mt == 0), stop=(mt == 1))
                        ccols = slice(wi * SEQ_TOK, (wi + 1) * SEQ_TOK)
                        nc.scalar.activation(chunkbuf[r0:r0 + VDIM, pt, ccols],
                                             avp[:VDIM, :], AF.Copy)
                        # arbitrary partition starts are DMA-only; DMA can't
                        # read PSUM, so take the sum row from chunkbuf (SBUF).
                        # gpsimd-initiated DMAs can cast bf16 -> f32.
                        nc.gpsimd.dma_start(out=srows8[h:h + 1, ccols],
                                            in_=chunkbuf[r0 + 32:r0 + 33, pt, ccols])
                self.finish_heads(chunkbuf, srows8, pools, 0)
                self.wo_residual(chunkbuf, w_o, c0 + wpair * 512, 0, pools)

    # ---------------- stage B/D: temporal attention ----------------
    def stage_temporal(self, sctx, w_q, w_k, w_v, w_o, tabq, tvrep):
        nc, tc = self.nc, self.tc
        from contextlib import ExitStack
        pools = self._mk_pools(sctx)
        qkvp = sctx.enter_context(tc.tile_pool(name="qkvT", bufs=1))
        spp = sctx.enter_context(tc.tile_pool(name="spT", bufs=2, space="PSUM"))
        avpp = sctx.enter_context(tc.tile_pool(name="avpT", bufs=1, space="PSUM"))
        rvpp = sctx.enter_context(tc.tile_pool(name="rvpT", bufs=1, space="PSUM"))
        epp = sctx.enter_context(tc.tile_pool(name="epT", bufs=2))

        nseq_c = self.chunk // T_LEN          # sequences per chunk
        ngrp_c = self.chunk // 128            # 8-seq groups per chunk
        # one chunkbuf for the whole stage: pad rows are zeroed once and the
        # finish multiply rewrites them as 0*recip = 0, so they stay zero
        chunkbuf = pools['attn'].tile([128, 4, self.chunk], BF16,
                                      tag="chunkbufT", bufs=1)
        nc.gpsimd.memset(chunkbuf, 0.0)
        nc.sync.dma_start(out=chunkbuf[105:106, 3, :],
                          in_=self.ones_row_bf[:1, :self.chunk])
        srows8 = pools['attn'].tile([8, self.chunk], F32,
                                    tag="srowsT", bufs=1)
        for c0 in range(0, self.ntok, self.chunk):
            qT = qkvp.tile([128, 4, self.chunk], BF16, tag="qT")
            kT = qkvp.tile([128, 4, self.chunk], BF16, tag="kT")
            vP = qkvp.tile([128, self.chunk // 128, 328], BF16, tag="vP")
            self.qkv_chunk(c0, pools, w_q, w_k, w_v, qT, kT, vP)

            # qaug[plane h//4, (h%4)*32+J, i*nseq_c + seq]
            #   = q_h[:, tok(seq,i)] . tabQ[:, i*16+J]
            qaug = qkvp.tile([128, 2, T_LEN * nseq_c], BF16, tag="qaug")
            i_per = 512 // nseq_c
            for plane in range(2):
                for r in range(T_LEN // i_per):
                    ps = spp.tile([128, 512], F32, tag="sp")
                    for ii in range(i_per):
                        i = r * i_per + ii
                        for hh in range(4):
                            h = plane * 4 + hh
                            pt, r0 = hrow(h)
                            nc.tensor.matmul(
                                ps[hh * 32: hh * 32 + 16,
                                   ii * nseq_c:(ii + 1) * nseq_c],
                                tabq[r0:r0 + DH, i * 16:(i + 1) * 16],
                                qT[r0:r0 + DH, pt, i::T_LEN],
                                start=True, stop=True,
                                tile_position=(r0, hh * 32))
                    for hh in range(4):
                        nc.scalar.activation(
                            qaug[hh * 32: hh * 32 + 16, plane,
                                 r * 512:(r + 1) * 512],
                            ps[hh * 32: hh * 32 + 16, :], AF.Copy)

            for h in range(HEADS):
                pt, r0 = hrow(h)
                qb = (h % 4) * 32      # qaug row base
                plane = h // 4
                ep = epp.tile([128, self.chunk], BF16, tag="ep")
                # scores in 4-group (512-col) batches: one mask add + one
                # exp per batch instead of per 128-col group
                for quad in range(ngrp_c // 4):
                    sp = spp.tile([128, 512], F32, tag="sp")
                    for g4 in range(4):
                        g = quad * 4 + g4
                        t0 = g * 128
                        qs = slice(g4 * 128, (g4 + 1) * 128)
                        nc.tensor.matmul(sp[:, qs],
                                         kT[r0:r0 + DH, pt, t0:t0 + 128],
                                         qT[r0:r0 + DH, pt, t0:t0 + 128],
                                         start=True, stop=False)
                        rhs = mkap(qaug, qb * qaug.ap[0][0]
                                   + plane * qaug.ap[1][0] + g * 8,
                                   [[qaug.ap[0][0], 16], [1, 8], [nseq_c, 16]])
                        nc.tensor.matmul(sp[:, qs], self.kaug[qb:qb + 16, :], rhs,
                                         start=False, stop=True,
                                         tile_position=(qb, 0))
                    nc.vector.tensor_tensor(out=sp, in0=sp, in1=self.mask4,
                                            op=ALU.add)
                    nc.scalar.activation(ep[:, quad * 512:(quad + 1) * 512],
                                         sp, AF.Exp, scale=SCALE)
                for quad in range(ngrp_c // 4):
                    avp = avpp.tile([128, 512], F32, tag="avp")
                    for g4 in range(4):
                        g = quad * 4 + g4
                        t0 = g * 128
                        nc.tensor.matmul(avp[r0:r0 + VDIM,
                                             g4 * 128:(g4 + 1) * 128],
                                         vP[:, g, h * VDIM: (h + 1) * VDIM],
                                         ep[:, t0:t0 + 128],
                                         start=True, stop=True,
                                         tile_position=(0, r0))
                    nc.scalar.activation(
                        chunkbuf[r0:r0 + VDIM, pt,
                                 quad * 512:(quad + 1) * 512],
                        avp[r0:r0 + VDIM, :], AF.Copy)
                # rel-v (writes 41 rows; the sum slot col of tvrep is zero)
                for rr in range(T_LEN // i_per):
                    rvp = rvpp.tile([128, 512], F32, tag="rvp")
                    for ii in range(i_per):
                        i = rr * i_per + ii
                        nc.tensor.matmul(rvp[r0:r0 + VDIM,
                                             ii * nseq_c:(ii + 1) * nseq_c],
                                         tvrep[:, i * VDIM:(i + 1) * VDIM],
                                         ep[:, i::T_LEN], start=True, stop=True,
                                         tile_position=(0, r0))
                    dst = mkap(chunkbuf, r0 * chunkbuf.ap[0][0]
                               + pt * chunkbuf.ap[1][0] + rr * i_per,
                               [[chunkbuf.ap[0][0], VDIM], [T_LEN, nseq_c],
                                [1, i_per]])
                    src_ = mkap(rvp, r0 * rvp.ap[0][0],
                                [[rvp.ap[0][0], VDIM], [1, nseq_c],
                                 [nseq_c, i_per]])
                    nc.vector.tensor_tensor(out=dst, in0=dst, in1=src_, op=ALU.add)
                nc.gpsimd.dma_start(out=srows8[h:h + 1, :],
                                    in_=chunkbuf[r0 + 32:r0 + 33, pt, :])
            for n512 in range(self.chunk // 512):
                self.finish_heads(chunkbuf, srows8, pools, n512)
                self.wo_residual(chunkbuf, w_o, c0, n512, pools)

    # ---------------- stage C: cross attention ----------------
    def stage_cross(self, sctx, w_q, w_k, w_v, w_o):
        nc, tc = self.nc, self.tc
        pools = self._mk_pools(sctx)
        qkvp = sctx.enter_context(tc.tile_pool(name="qkvC", bufs=2))
        kvp = sctx.enter_context(tc.tile_pool(name="kvC", bufs=1))
        spp = sctx.enter_context(tc.tile_pool(name="spC", bufs=2, space="PSUM"))
        avpp = sctx.enter_context(tc.tile_pool(name="avpC", bufs=2, space="PSUM"))
        epp = sctx.enter_context(tc.tile_pool(name="epC", bufs=2))

        # K/V from context (once)
        ctxB = kvp.tile([128, 6, NCTX], BF16, name="ctxB")
        nc.vector.tensor_copy(out=ctxB, in_=self.ctxT)
        kT2 = kvp.tile([128, 4, NCTX], BF16, name="kT2")
        vP2 = kvp.tile([128, 328], BF16, name="vP2")
        for mt in range(4):
            ps = pools['psum'].tile([128, 512], F32, tag="proj0")
            for k in range(6):
                nc.tensor.matmul(ps[:, :NCTX],
                                 w_k[:, k, mt * 128: mt * 128 + 128],
                                 ctxB[:, k, :],
                                 start=(k == 0), stop=(k == 5))
            nc.vector.tensor_copy(out=kT2[:, mt, :], in_=ps[:, :NCTX])
        ps = pools['psum'].tile([128, 512], F32, tag="proj1")
        for k in range(6):
            nc.tensor.matmul(ps[:NCTX, :328], ctxB[:, k, :], w_v[:, k, :328],
                             start=(k == 0), stop=(k == 5))
        nc.vector.tensor_copy(out=vP2[:NCTX, :], in_=ps[:NCTX, :328])
        onescols = mkap(vP2, 32, [[vP2.ap[0][0], NCTX], [VDIM, 8]])
        nc.gpsimd.memset(onescols, 1.0)

        for c0 in range(0, self.ntok, self.chunk):
            qT = qkvp.tile([128, 4, self.chunk], BF16, tag="qT")
            for n in range(self.chunk // 512):
                tok0 = c0 + n * 512
                nhat = self.ln_stage(tok0, 512, pools)
                for mt in range(4):
                    ps = pools['psum'].tile([128, 512], F32, tag=f"proj{mt % 2}")
                    for ki, (k, rows) in enumerate(self.KS_AUG):
                        nc.tensor.matmul(ps,
                                         w_q[:rows, k, mt * 128: mt * 128 + 128],
                                         nhat[:rows, k, :],
                                         start=(ki == 0), stop=(ki == 2))
                    nc.scalar.activation(qT[:, mt, n * 512:(n + 1) * 512],
                                         ps, AF.Copy)
            for n in range(self.chunk // 512):
                ns = slice(n * 512, (n + 1) * 512)
                chunkbuf = pools['attn'].tile([128, 4, 512], BF16, tag="chunkbuf")
                nc.vector.memset(chunkbuf, 0.0)
                nc.sync.dma_start(out=chunkbuf[105:106, 3, :],
                                  in_=self.ones_row_bf[:1, :512])
                srows8 = pools['attn'].tile([8, 512], F32, tag="srows8")
                for h in range(HEADS):
                    pt, r0 = hrow(h)
                    sp = spp.tile([128, 512], F32, tag="sp")
                    nc.tensor.matmul(sp[:NCTX, :], kT2[r0:r0 + DH, pt, :],
                                     qT[r0:r0 + DH, pt, ns],
                                     start=True, stop=True)
                    ep = epp.tile([128, 512], BF16, tag="ep")
                    nc.scalar.activation(ep[:NCTX, :], sp[:NCTX, :], AF.Exp, scale=SCALE)
                    avp = avpp.tile([48, 512], F32, tag="avp")
                    nc.tensor.matmul(avp[:VDIM, :],
                                     vP2[:NCTX, h * VDIM: (h + 1) * VDIM],
                                     ep[:NCTX, :], start=True, stop=True)
                    nc.scalar.activation(chunkbuf[r0:r0 + VDIM, pt, :],
                                         avp[:VDIM, :], AF.Copy)
                    nc.gpsimd.dma_start(out=srows8[h:h + 1, :],
                                        in_=chunkbuf[r0 + 32:r0 + 33, pt, :])
                self.finish_heads(chunkbuf, srows8, pools, 0)
                self.wo_residual(chunkbuf, w_o, c0 + n * 512, 0, pools)

    # ---------------- stage E: GEGLU FF ----------------
    def stage_ff(self, sctx, w1, w2):
        nc, tc = self.nc, self.tc
        pools = self._mk_pools(sctx)
        ffp = sctx.enter_context(tc.tile_pool(name="ffp", bufs=2))
        gpsum = sctx.enter_context(tc.tile_pool(name="gps", bufs=2, space="PSUM"))

        for n in range(self.ntok // 512):
            tok0 = n * 512
            nhat = self.ln_stage(tok0, 512, pools)
            ff = ffp.tile([128, 11, 512], BF16, tag="ff")
            for mt in range(10):
                aps = gpsum.tile([128, 512], F32, tag="apsum")
                gps = gpsum.tile([128, 512], F32, tag="gpsum")
                for ki, (k, rows) in enumerate(self.KS_AUG):
                    nc.tensor.matmul(aps,
                                     w1[:rows, k, mt * 128: mt * 128 + 128],
                                     nhat[:rows, k, :],
                                     start=(ki == 0), stop=(ki == 2))
                    nc.tensor.matmul(gps,
                                     w1[:rows, k, FF + mt * 128: FF + mt * 128 + 128],
                                     nhat[:rows, k, :],
                                     start=(ki == 0), stop=(ki == 2))
                gelu = pools['ln'].tile([128, 512], BF16, tag="gelu")
                if self.sim_gelu:
                    # CoreSim lacks Gelu: x*sigmoid(1.702x) stand-in, matched
                    # by the hostref flag. HW uses the real erf Gelu below.
                    nc.scalar.activation(gelu, gps, AF.Sigmoid, scale=1.702)
                    nc.vector.tensor_tensor(out=gelu, in0=gps, in1=gelu,
                                            op=ALU.mult)
                else:
                    nc.scalar.activation(gelu, gps, AF.Gelu)
                nc.vector.tensor_tensor(out=ff[:, mt, :], in0=aps, in1=gelu,
                                        op=ALU.mult)
            nc.gpsimd.memset(ff[0:1, 10, :], 1.0)
            for mt in range(3):
                mrows = 128 if mt < 2 else 64
                ps = pools['psum'].tile([128, 512], F32, tag=f"proj{mt % 2}")
                for k in range(10):
                    nc.tensor.matmul(ps[:mrows, :],
                                     w2[:, k, mt * 128: mt * 128 + mrows],
                                     ff[:, k, :], start=(k == 0), stop=False)
                nc.tensor.matmul(ps[:mrows, :],
                                 w2[0:1, 10, mt * 128: mt * 128 + mrows],
                                 ff[0:1, 10, :], start=False, stop=True)
                cols = slice(tok0, tok0 + 512)
                nc.vector.tensor_tensor(out=self.xT[:mrows, mt, cols],
                                        in0=ps[:mrows, :],
                                        in1=self.xT[:mrows, mt, cols], op=ALU.add)


# ----------------------------------------------------------------------------
# host entry point
# ----------------------------------------------------------------------------

_nc_cache = {}


def _get_nc(nwin=16, chunk_win=8, stages=('a1', 't1', 'a2', 't2', 'ff')):
    key = (nwin, chunk_win, stages)
    if key not in _nc_cache:
        _nc_cache[key] = Builder(nwin, chunk_win, stages).build(
            num_devices=NCORES)
    return _nc_cache[key]


def make_in_maps(inputs, nwin=16):
    x = np.asarray(inputs['x'], np.float32)
    context = np.asarray(inputs['context'], np.float32)
    wd = {k: np.asarray(v, np.float32) for k, v in inputs.items()
          if k not in ('x', 'context')}
    wt = prep_weights(wd)
    shards = shard_x(x, nwin)
    ncore = shards.shape[0]
    in_maps = []
    for c in range(ncore):
        bidx = (c * nwin) // (NH * NH)
        ctxT = _cmajor(np.ascontiguousarray(context[bidx].T), 768)  # [128,6,77]
        m = {'xT': np.ascontiguousarray(_cmajor(shards[c], 384), dtype=np.float32),
             'ctxT': np.ascontiguousarray(ctxT, dtype=np.float32)}
        m.update(wt)
        in_maps.append(m)
    return in_maps


def kernel(**inputs):
    nwin = 16
    nc = _get_nc(nwin)
    in_maps = make_in_maps(inputs, nwin)
    res = run_bass_kernel_spmd(nc, in_maps, list(range(NCORES)))
    outs = np.stack([r['yT'] for r in res.results])  # [8, 128, 3, ntok]
    # undo c-major padding: [8, 128, 3, ntok] -> [8, 384, ntok] -> [8, 320, ntok]
    outs = outs.transpose(0, 2, 1, 3).reshape(NCORES, 384, nwin * SEQ_TOK)[:, :D]
    return unshard_x(outs, nwin).astype(np.float32)



# revision 34
# speedup vs baseline: 1.2985x; 1.0665x over previous
# Trainium2 Bass kernel for nn_BasicTransformerBlockST (spatio-temporal
# transformer block: windowed spatial self-attention, two temporal
# self-attentions with relative-position bias + causal mask, cross-attention
# to a text context, and a GEGLU feed-forward).
#
# Sharding: data-parallel over the 128 (b, nh, nw) spatial windows -> 16
# windows x 4096 tokens per core; every stage (window attn / temporal attn /
# cross attn / FF) is closed under this shard, so no collectives are needed.
#
# Per-core layout: activations channel-major xT [C=320, ntok] resident in SBUF
# as [128, 3, ntok]; token order (window, spatial, t) makes temporal sequences
# contiguous 16-token runs. Softmax runs in transposed score space S^T[k, q]
# without max subtraction (logits are O(1)); the normalizer comes from an
# appended ones-column in V and is applied post-attention. Masked entries get
# -1e5 before exp and underflow to exactly 0, which makes the block-diagonal
# batched temporal attention exact. LayerNorm is folded: gamma/beta fold into
# the projection weights; the -mu*rstd and beta terms ride along as augmented
# contraction rows.
import numpy as np
import ml_dtypes

import concourse.bass as bass
import concourse.tile as tile
from concourse import bacc, mybir
from concourse.bass_utils import run_bass_kernel_spmd

F32 = mybir.dt.float32
F32R = mybir.dt.float32r
BF16 = mybir.dt.bfloat16
AF = mybir.ActivationFunctionType
ALU = mybir.AluOpType

D, CTX_DIM, HEADS, DH, T_LEN, WS, MAXREL, FF = 320, 768, 8, 40, 16, 4, 16, 1280
B, H, W = 2, 32, 32
NH = H // WS
NWIN = B * NH * NH          # 128 windows total
NCORES = 8
SEQ_TOK = T_LEN * WS * WS   # 256 tokens per window
SCALE = DH ** -0.5
NEG = -1e5
NCTX = 77
EPS = 1e-5

bfdt = ml_dtypes.bfloat16


# ----------------------------------------------------------------------------
# host-side data prep
# ----------------------------------------------------------------------------

def shard_x(x, win_per_core):
    xr = np.asarray(x, np.float32).reshape(B, D, T_LEN, NH, WS, NH, WS)
    xr = xr.transpose(0, 3, 5, 1, 4, 6, 2)          # B nh nw C wh ww T
    xr = xr.reshape(NWIN, D, WS * WS * T_LEN)       # win C (s t)
    ncore = NWIN // win_per_core
    xr = xr.reshape(ncore, win_per_core, D, WS * WS * T_LEN)
    xr = xr.transpose(0, 2, 1, 3).reshape(ncore, D, win_per_core * WS * WS * T_LEN)
    return np.ascontiguousarray(xr)


def unshard_x(shards, win_per_core):
    ncore = NWIN // win_per_core
    shards = np.asarray(shards, np.float32)
    xr = shards.reshape(ncore, D, win_per_core, WS * WS * T_LEN).transpose(0, 2, 1, 3)
    xr = xr.reshape(B, NH, NH, D, WS, WS, T_LEN)
    xr = xr.transpose(0, 3, 6, 1, 4, 2, 5)          # B C T nh wh nw ww
    return np.ascontiguousarray(xr.reshape(B, D, T_LEN, H, W))


def _cmajor(a, rows):
    """[rows_logical<=rows, cols] -> [128, rows/128, cols], zero padded."""
    out = np.zeros((rows, a.shape[1]), np.float32)
    out[: a.shape[0]] = a
    return np.ascontiguousarray(
        out.reshape(rows // 128, 128, a.shape[1]).transpose(1, 0, 2))


VDIM = DH + 1    # 41 per-head value columns; slot 32 is the ones column


def vslot(c):
    """map v-slot index c in [0,41) to head dim, or None for the ones slot."""
    if c == 32:
        return None
    return c if c < 32 else c - 1


def pad_v_cols(Wv):
    """[cin, 320] -> [cin, 328]: per-head 41 columns; slot 32 left zero
    (filled with ones on device for the softmax-denominator trick)."""
    cin = Wv.shape[0]
    out = np.zeros((cin, HEADS * VDIM), np.float32)
    for h in range(HEADS):
        for c in range(VDIM):
            d = vslot(c)
            if d is not None:
                out[:, h * VDIM + c] = Wv[:, h * DH + d]
    return out


def pad_head_cols(Wx):
    """[cin, 320] -> [cin, 512]: head h cols at h*64+[0,40), zeros between."""
    out = np.zeros((Wx.shape[0], 512), np.float32)
    for h in range(HEADS):
        out[:, h * 64: h * 64 + 40] = Wx[:, h * 40: (h + 1) * 40]
    return out


def prep_proj_w(Wraw, gamma, beta, extra_bias=None, pad_heads=False):
    """Augmented c-major projection weight [128, 3, dout]:
    rows 0..320 = W*gamma[:,None]; row (2,64) = colsum (pairs with -mu*rstd);
    row (2,65) = beta@Wg (+extra_bias)."""
    Wg = np.asarray(Wraw, np.float32) * np.asarray(gamma, np.float32)[:, None]
    if pad_heads:
        Wg = pad_head_cols(Wg)
    out = np.zeros((384, Wg.shape[1]), np.float32)
    out[:320] = Wg
    out[256 + 64] = Wg.sum(0)
    out[256 + 65] = np.asarray(beta, np.float32) @ Wg
    if extra_bias is not None:
        out[256 + 65] += np.asarray(extra_bias, np.float32)
    return _cmajor(out, 384)


def prep_wo(Wo, bo):
    """[320, 320] -> lhsT [128, 4, 320]: head h rows at h*64+c for v-slot c
    (zero at the sum slot c=32); bias rides at plane-3 row 105 (a pad row
    that the device keeps at constant 1.0)."""
    out = np.zeros((512, 320), np.float32)
    Wo = np.asarray(Wo, np.float32)
    for h in range(HEADS):
        for c in range(VDIM):
            d = vslot(c)
            if d is not None:
                out[h * 64 + c] = Wo[h * DH + d]
    out[3 * 128 + 105] = np.asarray(bo, np.float32)
    return _cmajor(out, 512)


def prep_tabq(table):
    """relk [33, 40] -> tabQ [128, 256]: col (i*16+J) holds table[J-i+16] in
    rows 0..40 AND a copy in rows 64..104 (so lhsT base matches q's base)."""
    out = np.zeros((128, 256), np.float32)
    t = np.asarray(table, np.float32)
    for i in range(T_LEN):
        for J in range(T_LEN):
            out[:40, i * 16 + J] = t[J - i + MAXREL]
            out[64:104, i * 16 + J] = t[J - i + MAXREL]
    return out


def prep_tvrep(table):
    """relv [33, 40] -> tvrep [128, 16*41]: for query pos i, col i*41+c
    (v-slot c; zero at c=32) row (s*16+j) holds table[j-i+16, dim(c)]."""
    out = np.zeros((128, T_LEN * VDIM), np.float32)
    t = np.asarray(table, np.float32)
    for i in range(T_LEN):
        for s in range(8):
            for j in range(T_LEN):
                for c in range(VDIM):
                    d = vslot(c)
                    if d is not None:
                        out[s * 16 + j, i * VDIM + c] = t[j - i + MAXREL, d]
    return out


def prep_kaug():
    """constant selector [128, 128]: rows b+J (for each base b in
    0/32/64/96) one at cols (s*16+J)."""
    out = np.zeros((128, 128), np.float32)
    for base in (0, 32, 64, 96):
        for s in range(8):
            for J in range(T_LEN):
                out[base + J, s * 16 + J] = 1.0
    return out


def prep_mask():
    """additive [128, 128]: row (s,j), col (s',i): 0 iff s==s' and j<=i."""
    m = np.full((128, 128), NEG, np.float32)
    for s in range(8):
        for j in range(T_LEN):
            m[s * 16 + j, s * 16 + j: (s + 1) * 16] = 0.0
    return m


def prep_selbc():
    """[8, 4, 128]: row h, plane pt: ones over head h's 64-row block."""
    out = np.zeros((8, 4, 128), np.float32)
    for h in range(8):
        out[h, h // 2, 64 * (h % 2): 64 * (h % 2) + 64] = 1.0
    return out


def prep_weights(wd):
    t = {}

    def bfc(x):
        return np.ascontiguousarray(np.asarray(x, np.float32).astype(bfdt))

    for nm, g, b in (('a1', wd['ln1_g'], wd['ln1_b']),
                     ('t1', wd['ln4_g'], wd['ln4_b']),
                     ('t2', wd['ln5_g'], wd['ln5_b'])):
        t[f'{nm}_wq'] = bfc(prep_proj_w(wd[f'{nm}_Wq'], g, b, pad_heads=True))
        t[f'{nm}_wk'] = bfc(prep_proj_w(wd[f'{nm}_Wk'], g, b, pad_heads=True))
        Wvp = pad_v_cols(np.asarray(wd[f'{nm}_Wv'], np.float32))
        t[f'{nm}_wv'] = bfc(prep_proj_w(Wvp, g, b))
        t[f'{nm}_wo'] = bfc(prep_wo(wd[f'{nm}_Wo'], wd[f'{nm}_bo']))
    t['a2_wq'] = bfc(prep_proj_w(wd['a2_Wq'], wd['ln2_g'], wd['ln2_b'],
                                 pad_heads=True))
    t['a2_wk'] = bfc(_cmajor(pad_head_cols(np.asarray(wd['a2_Wk'], np.float32)), 768))
    t['a2_wv'] = bfc(_cmajor(pad_v_cols(np.asarray(wd['a2_Wv'], np.float32)), 768))
    t['a2_wo'] = bfc(prep_wo(wd['a2_Wo'], wd['a2_bo']))
    t['ff_w1'] = bfc(prep_proj_w(wd['ff_W1'], wd['ln3_g'], wd['ln3_b'],
                                 extra_bias=wd['ff_b1']))
    W2aug = np.zeros((1408, 320), np.float32)
    W2aug[:1280] = np.asarray(wd['ff_W2'], np.float32)
    W2aug[1280] = np.asarray(wd['ff_b2'], np.float32)
    t['ff_w2'] = bfc(_cmajor(W2aug, 1408))
    t['t1_tabq'] = bfc(prep_tabq(wd['t1_relk']))
    t['t2_tabq'] = bfc(prep_tabq(wd['t2_relk']))
    t['t1_tvrep'] = bfc(prep_tvrep(wd['t1_relv']))
    t['t2_tvrep'] = bfc(prep_tvrep(wd['t2_relv']))
    t['kaug'] = bfc(prep_kaug())
    t['mask4'] = np.ascontiguousarray(np.tile(prep_mask(), (1, 4)))
    t['selbc'] = bfc(prep_selbc())
    return t


WEIGHT_SHAPES = {}
for _s in ('a1', 't1', 't2'):
    WEIGHT_SHAPES.update({f'{_s}_wq': ([128, 3, 512], BF16),
                          f'{_s}_wk': ([128, 3, 512], BF16),
                          f'{_s}_wv': ([128, 3, 328], BF16),
                          f'{_s}_wo': ([128, 4, 320], BF16)})
WEIGHT_SHAPES.update({
    'a2_wq': ([128, 3, 512], BF16), 'a2_wk': ([128, 6, 512], BF16),
    'a2_wv': ([128, 6, 328], BF16), 'a2_wo': ([128, 4, 320], BF16),
    'ff_w1': ([128, 3, 2560], BF16), 'ff_w2': ([128, 11, 320], BF16),
    't1_tabq': ([128, 256], BF16), 't2_tabq': ([128, 256], BF16),
    't1_tvrep': ([128, 656], BF16), 't2_tvrep': ([128, 656], BF16),
    'kaug': ([128, 128], BF16), 'mask4': ([128, 512], F32),
    'selbc': ([8, 4, 128], BF16),
})

STAGE_WEIGHTS = {
    'a1': ['a1_wq', 'a1_wk', 'a1_wv', 'a1_wo'],
    't1': ['t1_wq', 't1_wk', 't1_wv', 't1_wo', 't1_tabq', 't1_tvrep'],
    'a2': ['a2_wq', 'a2_wk', 'a2_wv', 'a2_wo'],
    't2': ['t2_wq', 't2_wk', 't2_wv', 't2_wo', 't2_tabq', 't2_tvrep'],
    'ff': ['ff_w1', 'ff_w2'],
}


# ----------------------------------------------------------------------------
# device kernel builder
# ----------------------------------------------------------------------------

def hrow(h):
    """(ptile, row0) of head h in the head-padded 512-row q/k layout."""
    return h // 2, (h % 2) * 64


def mkap(t, extra_off, dims):
    return bass.AP(tensor=t.tensor, offset=t.offset + extra_off, ap=[list(d) for d in dims])


class Builder:
    def __init__(self, nwin=16, chunk_win=4, stages=('a1', 't1', 'a2', 't2', 'ff'),
                 sim_gelu=False):
        self.sim_gelu = sim_gelu
        self.nwin = nwin
        self.ntok = nwin * SEQ_TOK
        self.chunk = min(chunk_win * SEQ_TOK, self.ntok)   # tokens per chunk
        self.stages = stages

    def build(self, num_devices=1):
        nc = bacc.Bacc("TRN2", target_bir_lowering=False, debug=False,
                       num_devices=num_devices)
        self.nc = nc
        dram = {}
        dram['xT'] = nc.declare_dram_parameter('xT', [128, 3, self.ntok], BF16,
                                               isOutput=False)
        dram['ctxT'] = nc.declare_dram_parameter('ctxT', [128, 6, NCTX], F32,
                                                 isOutput=False)
        for nm, (shp, dt) in WEIGHT_SHAPES.items():
            dram[nm] = nc.declare_dram_parameter(nm, list(shp), dt, isOutput=False)
        out_yT = nc.declare_dram_parameter('yT', [128, 3, self.ntok], BF16,
                                           isOutput=True)
        if getattr(self, 'debug_taps', False):
            dram['dbg_q'] = nc.declare_dram_parameter(
                'dbg_q', [128, 4, self.chunk], BF16, isOutput=True)
            dram['dbg_n'] = nc.declare_dram_parameter(
                'dbg_n', [128, 3, 512], BF16, isOutput=True)
            dram['dbg_cb'] = nc.declare_dram_parameter(
                'dbg_cb', [128, 5, 512], BF16, isOutput=True)
            dram['dbg_s'] = nc.declare_dram_parameter(
                'dbg_s', [128, 4, 512], F32, isOutput=True)
        self.dram = dram
        with tile.TileContext(nc) as tc:
            self.tc = tc
            self._emit(out_yT)
        nc.compile()
        return nc

    # ---------------- helpers ----------------
    def _emit(self, out_yT):
        from contextlib import ExitStack
        nc, tc = self.nc, self.tc
        with ExitStack() as ctx:
            resid = ctx.enter_context(tc.tile_pool(name="resid", bufs=1))
            consts = ctx.enter_context(tc.tile_pool(name="consts", bufs=1))

            xT = resid.tile([128, 3, self.ntok], BF16)
            nc.sync.dma_start(out=xT, in_=self.dram['xT'][:])
            self.xT = xT

            self.ones_col = consts.tile([128, 1], F32, name="onescol")
            nc.vector.memset(self.ones_col, 1.0)
            self.ones_col_bf = consts.tile([128, 1], BF16, name="onescolbf")
            nc.vector.memset(self.ones_col_bf, 1.0)
            self.eps_col = consts.tile([128, 1], F32, name="epscol")
            nc.vector.memset(self.eps_col, EPS)
            self.ones_row_bf = consts.tile([1, 2048], BF16, name="onesrowbf")
            nc.vector.memset(self.ones_row_bf, 1.0)
            self.mask4 = consts.tile([128, 512], F32, name="mask4c")
            nc.sync.dma_start(out=self.mask4, in_=self.dram['mask4'][:])
            self.kaug = consts.tile([128, 128], BF16, name="kaugc")
            nc.sync.dma_start(out=self.kaug, in_=self.dram['kaug'][:])
            self.selbc = consts.tile([8, 4, 128], BF16, name="selbcc")
            nc.sync.dma_start(out=self.selbc, in_=self.dram['selbc'][:])
            # context stays resident (tiny)
            self.ctxT = consts.tile([128, 6, NCTX], F32, name="ctxTc")
            nc.sync.dma_start(out=self.ctxT, in_=self.dram['ctxT'][:])

            for st in self.stages:
                with ExitStack() as sctx:
                    wpool = sctx.enter_context(
                        tc.tile_pool(name=f"w_{st}", bufs=1))
                    w = {}
                    for nm in STAGE_WEIGHTS[st]:
                        shp, dt = WEIGHT_SHAPES[nm]
                        w[nm] = wpool.tile(list(shp), dt, name=f"sb_{nm}")
                        nc.sync.dma_start(out=w[nm], in_=self.dram[nm][:])
                    if st == 'a1':
                        self.stage_window(sctx, w['a1_wq'], w['a1_wk'],
                                          w['a1_wv'], w['a1_wo'])
                    elif st in ('t1', 't2'):
                        self.stage_temporal(sctx, w[f'{st}_wq'], w[f'{st}_wk'],
                                            w[f'{st}_wv'], w[f'{st}_wo'],
                                            w[f'{st}_tabq'], w[f'{st}_tvrep'])
                    elif st == 'a2':
                        self.stage_cross(sctx, w['a2_wq'], w['a2_wk'],
                                         w['a2_wv'], w['a2_wo'])
                    elif st == 'ff':
                        self.stage_ff(sctx, w['ff_w1'], w['ff_w2'])

            nc.sync.dma_start(out=out_yT[:], in_=xT)

    def ln_stage(self, tok0, ncols, pools):
        """nhat [128, 3, ncols] bf16: rows 0..320 = (x-mu)*rstd;
        row (2,64) = -mu*rstd; row (2,65) = 1.0."""
        nc = self.nc
        xT = self.xT
        stat_psum = pools['stat_psum'].tile([33, 512], F32, tag="statp")
        sq = pools['ln'].tile([128, 3, 512], BF16, tag="lnsq")
        cols = slice(tok0, tok0 + ncols)
        # xT is bf16, so the stat matmuls stream it directly; plane-2 pad
        # rows are zero, so summing the full 128 rows changes nothing
        nc.scalar.square(sq[:, :, :ncols], xT[:, :, cols])
        for k in range(3):
            nc.tensor.matmul(stat_psum[0:1, :ncols],
                             self.ones_col_bf[:128],
                             xT[:, k, cols],
                             start=(k == 0), stop=(k == 2))
            nc.tensor.matmul(stat_psum[32:33, :ncols],
                             self.ones_col_bf[:128],
                             sq[:, k, :ncols],
                             start=(k == 0), stop=(k == 2))
        # single-row stat tiles, all at partition 0 (walrus requires equal
        # SBUF start partitions within one TensorTensor op)
        st_mu = pools['ln'].tile([1, 512], F32, tag="lnmu")
        st_ex2 = pools['ln'].tile([1, 512], F32, tag="lnex2")
        st_rstd = pools['ln'].tile([1, 512], F32, tag="lnrstd")
        st_murstd = pools['ln'].tile([1, 512], F32, tag="lnmurstd")
        nc.vector.tensor_scalar_mul(st_mu[:, :ncols], stat_psum[0:1, :ncols], 1.0 / D)
        nc.vector.tensor_scalar_mul(st_ex2[:, :ncols], stat_psum[32:33, :ncols], 1.0 / D)
        nc.vector.tensor_tensor(out=st_rstd[:, :ncols], in0=st_mu[:, :ncols],
                                in1=st_mu[:, :ncols], op=ALU.mult)
        nc.vector.tensor_tensor(out=st_rstd[:, :ncols], in0=st_ex2[:, :ncols],
                                in1=st_rstd[:, :ncols], op=ALU.subtract)
        nc.scalar.activation(st_rstd[:, :ncols], st_rstd[:, :ncols], AF.Sqrt,
                             bias=self.eps_col[:1])
        st_rscr = pools['ln'].tile([1, 512], F32, tag="lnrscr")
        nc.vector.reciprocal_approx_accurate(st_rstd[:, :ncols],
                                             st_rstd[:, :ncols],
                                             st_rscr[:, :ncols])
        nc.vector.tensor_tensor(out=st_murstd[:, :ncols], in0=st_mu[:, :ncols],
                                in1=st_rstd[:, :ncols], op=ALU.mult)
        nc.vector.tensor_scalar_mul(st_murstd[:, :ncols], st_murstd[:, :ncols], -1.0)
        rstd_b = pools['ln'].tile([128, 512], F32, tag="lnrstdb")
        nc.gpsimd.partition_broadcast(rstd_b[:, :ncols], st_rstd[:, :ncols])
        nhat = pools['nhat'].tile([128, 3, 512], BF16, tag="nhat")
        for k in range(3):
            rows = 128 if k < 2 else 64
            nc.vector.tensor_tensor(out=nhat[:rows, k, :ncols],
                                    in0=xT[:rows, k, cols],
                                    in1=rstd_b[:rows, :ncols], op=ALU.mult)
        nhat_mr = pools['ln'].tile([1, 512], BF16, tag="lnmrbf")
        nc.vector.tensor_copy(out=nhat_mr[:, :ncols], in_=st_murstd[:, :ncols])
        # aug rows via DMA (arbitrary partition starts are DMA-only)
        nc.sync.dma_start(out=nhat[64:65, 2, :ncols], in_=nhat_mr[:1, :ncols])
        nc.sync.dma_start(out=nhat[65:66, 2, :ncols],
                          in_=self.ones_row_bf[:1, :ncols])
        return nhat

    KS_AUG = ((0, 128), (1, 128), (2, 66))
    KS_NOAUG = ((0, 128), (1, 128), (2, 64))

    def qkv_chunk(self, c0, pools, w_q, w_k, w_v, qT, kT, vP):
        """LN + q/k/v for tokens [c0, c0+chunk): qT,kT [128,4,chunk] bf16
        c-major head-padded; vP [128, chunk/128, 328] bf16 token-major."""
        nc = self.nc
        ntt = self.chunk // 512
        for n in range(ntt):
            tok0 = c0 + n * 512
            nhat = self.ln_stage(tok0, 512, pools)
            if getattr(self, 'debug_taps', False) and tok0 == 0:
                nc.sync.dma_start(out=self.dram['dbg_n'][:], in_=nhat)
            for wsb, dst, eng in ((w_q, qT, 'act'), (w_k, kT, 'dve')):
                for mt in range(4):
                    ps = pools['psum'].tile([128, 512], F32, tag=f"proj{mt % 2}")
                    for ki, (k, rows) in enumerate(self.KS_AUG):
                        nc.tensor.matmul(ps,
                                         wsb[:rows, k, mt * 128: mt * 128 + 128],
                                         nhat[:rows, k, :],
                                         start=(ki == 0), stop=(ki == 2))
                    dcols = slice(n * 512, (n + 1) * 512)
                    if eng == 'act':
                        nc.scalar.activation(dst[:, mt, dcols], ps, AF.Copy)
                    else:
                        nc.vector.tensor_copy(out=dst[:, mt, dcols], in_=ps)
            for m in range(4):
                ps = pools['psum'].tile([128, 512], F32, tag=f"proj{m % 2}")
                for ki, (k, rows) in enumerate(self.KS_AUG):
                    nc.tensor.matmul(ps[:, :328],
                                     nhat[:rows, k, m * 128:(m + 1) * 128],
                                     w_v[:rows, k, :328],
                                     start=(ki == 0), stop=(ki == 2))
                blk = n * 4 + m
                if m % 2 == 0:
                    nc.scalar.activation(vP[:, blk, :328], ps[:, :328], AF.Copy)
                else:
                    nc.vector.tensor_copy(out=vP[:, blk, :328], in_=ps[:, :328])
        # ones columns: vP[:, :, 32::41]
        onescols = mkap(vP, 32, [[vP.ap[0][0], 128],
                                 [328, self.chunk // 128], [41, 8]])
        nc.gpsimd.memset(onescols, 1.0)

    def finish_heads(self, chunkbuf, srows8, pools, n512):
        """normalize chunkbuf[:, :4, cs] by per-head recips.
        srows8 [8, >=512] f32: head h sum at row h. One reciprocal over the
        8 used rows, then a K=8 selector matmul broadcasts each head's recip
        across its 64-row block, one plane at a time."""
        nc = self.nc
        cs = slice(n512 * 512, (n512 + 1) * 512)
        recf = pools['attn'].tile([8, 512], F32, tag="recipf")
        nc.vector.reciprocal_approx_fast(recf, srows8[:, cs])
        rec16 = pools['attn'].tile([8, 512], BF16, tag="rec16")
        nc.vector.tensor_copy(out=rec16, in_=recf)
        for pt in range(4):
            bps = pools['bcast_psum'].tile([128, 512], F32, tag="sp")
            nc.tensor.matmul(bps, self.selbc[:, pt, :], rec16,
                             start=True, stop=True)
            # rows 0..104 only: row 105 of plane 3 is the constant bias-ones
            # row; pad rows 41..63 are zero so the multiply keeps them zero
            nc.vector.tensor_tensor(out=chunkbuf[:105, pt, cs],
                                    in0=chunkbuf[:105, pt, cs],
                                    in1=bps[:105, :], op=ALU.mult)

    def wo_residual(self, chunkbuf, w_o, c0, n512, pools):
        """xT[:, :, cols] += Wo_pad^T @ chunkbuf-slice (+bo via the constant
        ones row at plane 3 row 105)."""
        nc = self.nc
        cs = slice(n512 * 512, (n512 + 1) * 512)
        xcols = slice(c0 + n512 * 512, c0 + (n512 + 1) * 512)
        for mt in range(3):
            mrows = 128 if mt < 2 else 64
            ps = pools['psum'].tile([128, 512], F32, tag=f"proj{mt % 2}")
            for k in range(4):
                nc.tensor.matmul(ps[:mrows, :],
                                 w_o[:, k, mt * 128: mt * 128 + mrows],
                                 chunkbuf[:, k, cs],
                                 start=(k == 0), stop=(k == 3))
            nc.vector.tensor_tensor(out=self.xT[:mrows, mt, xcols],
                                    in0=ps[:mrows, :],
                                    in1=self.xT[:mrows, mt, xcols], op=ALU.add)

    def _mk_pools(self, sctx, extra=()):
        from contextlib import ExitStack
        tc = self.tc
        pools = {
            'ln': sctx.enter_context(tc.tile_pool(name="lnp", bufs=2)),
            'nhat': sctx.enter_context(tc.tile_pool(name="nhatp", bufs=3)),
            'psum': sctx.enter_context(tc.tile_pool(name="projps", bufs=1, space="PSUM")),
            'stat_psum': sctx.enter_context(tc.tile_pool(name="statps", bufs=1, space="PSUM")),
            'bcast_psum': sctx.enter_context(tc.tile_pool(name="bcps", bufs=1, space="PSUM")),
            'attn': sctx.enter_context(tc.tile_pool(name="attnp", bufs=2)),
        }
        return pools

    # ---------------- stage A: window attention ----------------
    def stage_window(self, sctx, w_q, w_k, w_v, w_o):
        nc, tc = self.nc, self.tc
        pools = self._mk_pools(sctx)
        qkvp = sctx.enter_context(tc.tile_pool(name="qkvA", bufs=1))
        spp = sctx.enter_context(tc.tile_pool(name="spA", bufs=2, space="PSUM"))
        avpp = sctx.enter_context(tc.tile_pool(name="avpA", bufs=2, space="PSUM"))
        epp = sctx.enter_context(tc.tile_pool(name="epA", bufs=2))

        for c0 in range(0, self.ntok, self.chunk):
            qT = qkvp.tile([128, 4, self.chunk], BF16, tag="qT")
            kT = qkvp.tile([128, 4, self.chunk], BF16, tag="kT")
            vP = qkvp.tile([128, self.chunk // 128, 328], BF16, tag="vP")
            self.qkv_chunk(c0, pools, w_q, w_k, w_v, qT, kT, vP)
            for wpair in range(self.chunk // 512):
                chunkbuf = pools['attn'].tile([128, 4, 512], BF16, tag="chunkbuf")
                nc.vector.memset(chunkbuf, 0.0)
                nc.sync.dma_start(out=chunkbuf[105:106, 3, :],
                                  in_=self.ones_row_bf[:1, :512])
                srows8 = pools['attn'].tile([8, 512], F32, tag="srows8")
                for wi in range(2):
                    t0 = wpair * 512 + wi * SEQ_TOK
                    ep = epp.tile([128, 2, SEQ_TOK], BF16, tag="ep")
                    for h in range(HEADS):
                        pt, r0 = hrow(h)
                        sp = spp.tile([128, 2, SEQ_TOK], F32, tag="sp")
                        for mt in range(2):
                            nc.tensor.matmul(
                                sp[:, mt, :],
                                kT[r0:r0 + DH, pt, t0 + mt * 128: t0 + (mt + 1) * 128],
                                qT[r0:r0 + DH, pt, t0: t0 + SEQ_TOK],
                                start=True, stop=True)
                        nc.scalar.activation(ep, sp, AF.Exp, scale=SCALE)
                        avp = avpp.tile([48, SEQ_TOK], F32, tag="avp")
                        for mt in range(2):
                            nc.tensor.matmul(
                                avp[:VDIM, :],
                                vP[:, (t0 // 128) + mt, h * VDIM: (h + 1) * VDIM],
                                ep[:, mt, :],
                                start=(mt == 0), stop=(mt == 1))
                        ccols = slice(wi * SEQ_TOK, (wi + 1) * SEQ_TOK)
                        nc.scalar.activation(chunkbuf[r0:r0 + VDIM, pt, ccols],
                                             avp[:VDIM, :], AF.Copy)
                        # arbitrary partition starts are DMA-only; DMA can't
                        # read PSUM, so take the sum row from chunkbuf (SBUF).
                        # gpsimd-initiated DMAs can cast bf16 -> f32.
                        nc.gpsimd.dma_start(out=srows8[h:h + 1, ccols],
                                            in_=chunkbuf[r0 + 32:r0 + 33, pt, ccols])
                self.finish_heads(chunkbuf, srows8, pools, 0)
                self.wo_residual(chunkbuf, w_o, c0 + wpair * 512, 0, pools)

    # ---------------- stage B/D: temporal attention ----------------
    def stage_temporal(self, sctx, w_q, w_k, w_v, w_o, tabq, tvrep):
        nc, tc = self.nc, self.tc
        from contextlib import ExitStack
        pools = self._mk_pools(sctx)
        qkvp = sctx.enter_context(tc.tile_pool(name="qkvT", bufs=1))
        spp = sctx.enter_context(tc.tile_pool(name="spT", bufs=2, space="PSUM"))
        avpp = sctx.enter_context(tc.tile_pool(name="avpT", bufs=1, space="PSUM"))
        rvpp = sctx.enter_context(tc.tile_pool(name="rvpT", bufs=1, space="PSUM"))
        epp = sctx.enter_context(tc.tile_pool(name="epT", bufs=3))

        nseq_c = self.chunk // T_LEN          # sequences per chunk
        ngrp_c = self.chunk // 128            # 8-seq groups per chunk
        # one chunkbuf for the whole stage: pad rows are zeroed once and the
        # finish multiply rewrites them as 0*recip = 0, so they stay zero
        chunkbuf = pools['attn'].tile([128, 4, self.chunk], BF16,
                                      tag="chunkbufT", bufs=1)
        nc.gpsimd.memset(chunkbuf, 0.0)
        nc.sync.dma_start(out=chunkbuf[105:106, 3, :],
                          in_=self.ones_row_bf[:1, :self.chunk])
        srows8 = pools['attn'].tile([8, self.chunk], F32,
                                    tag="srowsT", bufs=1)
        for c0 in range(0, self.ntok, self.chunk):
            qT = qkvp.tile([128, 4, self.chunk], BF16, tag="qT")
            kT = qkvp.tile([128, 4, self.chunk], BF16, tag="kT")
            vP = qkvp.tile([128, self.chunk // 128, 328], BF16, tag="vP")
            self.qkv_chunk(c0, pools, w_q, w_k, w_v, qT, kT, vP)

            # qaug[plane h//4, (h%4)*32+J, i*nseq_c + seq]
            #   = q_h[:, tok(seq,i)] . tabQ[:, i*16+J]
            qaug = qkvp.tile([128, 2, T_LEN * nseq_c], BF16, tag="qaug")
            i_per = 512 // nseq_c
            for plane in range(2):
                for r in range(T_LEN // i_per):
                    ps = spp.tile([128, 512], F32, tag="sp")
                    for ii in range(i_per):
                        i = r * i_per + ii
                        for hh in range(4):
                            h = plane * 4 + hh
                            pt, r0 = hrow(h)
                            nc.tensor.matmul(
                                ps[hh * 32: hh * 32 + 16,
                                   ii * nseq_c:(ii + 1) * nseq_c],
                                tabq[r0:r0 + DH, i * 16:(i + 1) * 16],
                                qT[r0:r0 + DH, pt, i::T_LEN],
                                start=True, stop=True,
                                tile_position=(r0, hh * 32))
                    for hh in range(4):
                        nc.scalar.activation(
                            qaug[hh * 32: hh * 32 + 16, plane,
                                 r * 512:(r + 1) * 512],
                            ps[hh * 32: hh * 32 + 16, :], AF.Copy)

            for h in range(HEADS):
                pt, r0 = hrow(h)
                qb = (h % 4) * 32      # qaug row base
                plane = h // 4
                ep = epp.tile([128, self.chunk], BF16, tag="ep")
                # scores in 4-group (512-col) batches: one mask add + one
                # exp per batch instead of per 128-col group
                for quad in range(ngrp_c // 4):
                    sp = spp.tile([128, 512], F32, tag="sp")
                    for g4 in range(4):
                        g = quad * 4 + g4
                        t0 = g * 128
                        qs = slice(g4 * 128, (g4 + 1) * 128)
                        nc.tensor.matmul(sp[:, qs],
                                         kT[r0:r0 + DH, pt, t0:t0 + 128],
                                         qT[r0:r0 + DH, pt, t0:t0 + 128],
                                         start=True, stop=False)
                        rhs = mkap(qaug, qb * qaug.ap[0][0]
                                   + plane * qaug.ap[1][0] + g * 8,
                                   [[qaug.ap[0][0], 16], [1, 8], [nseq_c, 16]])
                        nc.tensor.matmul(sp[:, qs], self.kaug[qb:qb + 16, :], rhs,
                                         start=False, stop=True,
                                         tile_position=(qb, 0))
                    nc.vector.tensor_tensor(out=sp, in0=sp, in1=self.mask4,
                                            op=ALU.add)
                    nc.scalar.activation(ep[:, quad * 512:(quad + 1) * 512],
                                         sp, AF.Exp, scale=SCALE)
                for quad in range(ngrp_c // 4):
                    avp = avpp.tile([128, 512], F32, tag="avp")
                    for g4 in range(4):
                        g = quad * 4 + g4
                        t0 = g * 128
                        nc.tensor.matmul(avp[r0:r0 + VDIM,
                                             g4 * 128:(g4 + 1) * 128],
                                         vP[:, g, h * VDIM: (h + 1) * VDIM],
                                         ep[:, t0:t0 + 128],
                                         start=True, stop=True,
                                         tile_position=(0, r0))
                    nc.scalar.activation(
                        chunkbuf[r0:r0 + VDIM, pt,
                                 quad * 512:(quad + 1) * 512],
                        avp[r0:r0 + VDIM, :], AF.Copy)
                # rel-v (writes 41 rows; the sum slot col of tvrep is zero)
                for rr in range(T_LEN // i_per):
                    rvp = rvpp.tile([128, 512], F32, tag="rvp")
                    for ii in range(i_per):
                        i = rr * i_per + ii
                        nc.tensor.matmul(rvp[r0:r0 + VDIM,
                                             ii * nseq_c:(ii + 1) * nseq_c],
                                         tvrep[:, i * VDIM:(i + 1) * VDIM],
                                         ep[:, i::T_LEN], start=True, stop=True,
                                         tile_position=(0, r0))
                    dst = mkap(chunkbuf, r0 * chunkbuf.ap[0][0]
                               + pt * chunkbuf.ap[1][0] + rr * i_per,
                               [[chunkbuf.ap[0][0], VDIM], [T_LEN, nseq_c],
                                [1, i_per]])
                    src_ = mkap(rvp, r0 * rvp.ap[0][0],
                                [[rvp.ap[0][0], VDIM], [1, nseq_c],
                                 [nseq_c, i_per]])
                    nc.vector.tensor_tensor(out=dst, in0=dst, in1=src_, op=ALU.add)
                nc.gpsimd.dma_start(out=srows8[h:h + 1, :],
                                    in_=chunkbuf[r0 + 32:r0 + 33, pt, :])
            for n512 in range(self.chunk // 512):
                self.finish_heads(chunkbuf, srows8, pools, n512)
                self.wo_residual(chunkbuf, w_o, c0, n512, pools)

    # ---------------- stage C: cross attention ----------------
    def stage_cross(self, sctx, w_q, w_k, w_v, w_o):
        nc, tc = self.nc, self.tc
        pools = self._mk_pools(sctx)
        qkvp = sctx.enter_context(tc.tile_pool(name="qkvC", bufs=2))
        kvp = sctx.enter_context(tc.tile_pool(name="kvC", bufs=1))
        spp = sctx.enter_context(tc.tile_pool(name="spC", bufs=2, space="PSUM"))
        avpp = sctx.enter_context(tc.tile_pool(name="avpC", bufs=2, space="PSUM"))
        epp = sctx.enter_context(tc.tile_pool(name="epC", bufs=2))

        # K/V from context (once)
        ctxB = kvp.tile([128, 6, NCTX], BF16, name="ctxB")
        nc.vector.tensor_copy(out=ctxB, in_=self.ctxT)
        kT2 = kvp.tile([128, 4, NCTX], BF16, name="kT2")
        vP2 = kvp.tile([128, 328], BF16, name="vP2")
        for mt in range(4):
            ps = pools['psum'].tile([128, 512], F32, tag="proj0")
            for k in range(6):
                nc.tensor.matmul(ps[:, :NCTX],
                                 w_k[:, k, mt * 128: mt * 128 + 128],
                                 ctxB[:, k, :],
                                 start=(k == 0), stop=(k == 5))
            nc.vector.tensor_copy(out=kT2[:, mt, :], in_=ps[:, :NCTX])
        ps = pools['psum'].tile([128, 512], F32, tag="proj1")
        for k in range(6):
            nc.tensor.matmul(ps[:NCTX, :328], ctxB[:, k, :], w_v[:, k, :328],
                             start=(k == 0), stop=(k == 5))
        nc.vector.tensor_copy(out=vP2[:NCTX, :], in_=ps[:NCTX, :328])
        onescols = mkap(vP2, 32, [[vP2.ap[0][0], NCTX], [VDIM, 8]])
        nc.gpsimd.memset(onescols, 1.0)

        for c0 in range(0, self.ntok, self.chunk):
            qT = qkvp.tile([128, 4, self.chunk], BF16, tag="qT")
            for n in range(self.chunk // 512):
                tok0 = c0 + n * 512
                nhat = self.ln_stage(tok0, 512, pools)
                for mt in range(4):
                    ps = pools['psum'].tile([128, 512], F32, tag=f"proj{mt % 2}")
                    for ki, (k, rows) in enumerate(self.KS_AUG):
                        nc.tensor.matmul(ps,
                                         w_q[:rows, k, mt * 128: mt * 128 + 128],
                                         nhat[:rows, k, :],
                                         start=(ki == 0), stop=(ki == 2))
                    nc.scalar.activation(qT[:, mt, n * 512:(n + 1) * 512],
                                         ps, AF.Copy)
            for n in range(self.chunk // 512):
                ns = slice(n * 512, (n + 1) * 512)
                chunkbuf = pools['attn'].tile([128, 4, 512], BF16, tag="chunkbuf")
                nc.vector.memset(chunkbuf, 0.0)
                nc.sync.dma_start(out=chunkbuf[105:106, 3, :],
                                  in_=self.ones_row_bf[:1, :512])
                srows8 = pools['attn'].tile([8, 512], F32, tag="srows8")
                for h in range(HEADS):
                    pt, r0 = hrow(h)
                    sp = spp.tile([128, 512], F32, tag="sp")
                    nc.tensor.matmul(sp[:NCTX, :], kT2[r0:r0 + DH, pt, :],
                                     qT[r0:r0 + DH, pt, ns],
                                     start=True, stop=True)
                    ep = epp.tile([128, 512], BF16, tag="ep")
                    nc.scalar.activation(ep[:NCTX, :], sp[:NCTX, :], AF.Exp, scale=SCALE)
                    avp = avpp.tile([48, 512], F32, tag="avp")
                    nc.tensor.matmul(avp[:VDIM, :],
                                     vP2[:NCTX, h * VDIM: (h + 1) * VDIM],
                                     ep[:NCTX, :], start=True, stop=True)
                    nc.scalar.activation(chunkbuf[r0:r0 + VDIM, pt, :],
                                         avp[:VDIM, :], AF.Copy)
                    nc.gpsimd.dma_start(out=srows8[h:h + 1, :],
                                        in_=chunkbuf[r0 + 32:r0 + 33, pt, :])
                self.finish_heads(chunkbuf, srows8, pools, 0)
                self.wo_residual(chunkbuf, w_o, c0 + n * 512, 0, pools)

    # ---------------- stage E: GEGLU FF ----------------
    def stage_ff(self, sctx, w1, w2):
        nc, tc = self.nc, self.tc
        pools = self._mk_pools(sctx)
        ffp = sctx.enter_context(tc.tile_pool(name="ffp", bufs=2))
        gpsum = sctx.enter_context(tc.tile_pool(name="gps", bufs=2, space="PSUM"))

        for n in range(self.ntok // 512):
            tok0 = n * 512
            nhat = self.ln_stage(tok0, 512, pools)
            ff = ffp.tile([128, 11, 512], BF16, tag="ff")
            for mt in range(10):
                aps = gpsum.tile([128, 512], F32, tag="apsum")
                gps = gpsum.tile([128, 512], F32, tag="gpsum")
                for ki, (k, rows) in enumerate(self.KS_AUG):
                    nc.tensor.matmul(aps,
                                     w1[:rows, k, mt * 128: mt * 128 + 128],
                                     nhat[:rows, k, :],
                                     start=(ki == 0), stop=(ki == 2))
                    nc.tensor.matmul(gps,
                                     w1[:rows, k, FF + mt * 128: FF + mt * 128 + 128],
                                     nhat[:rows, k, :],
                                     start=(ki == 0), stop=(ki == 2))
                gelu = pools['ln'].tile([128, 512], BF16, tag="gelu")
                if self.sim_gelu:
                    # CoreSim lacks Gelu: x*sigmoid(1.702x) stand-in, matched
                    # by the hostref flag. HW uses the real erf Gelu below.
                    nc.scalar.activation(gelu, gps, AF.Sigmoid, scale=1.702)
                    nc.vector.tensor_tensor(out=gelu, in0=gps, in1=gelu,
                                            op=ALU.mult)
                else:
                    nc.scalar.activation(gelu, gps, AF.Gelu)
                nc.vector.tensor_tensor(out=ff[:, mt, :], in0=aps, in1=gelu,
                                        op=ALU.mult)
            nc.gpsimd.memset(ff[0:1, 10, :], 1.0)
            for mt in range(3):
                mrows = 128 if mt < 2 else 64
                ps = pools['psum'].tile([128, 512], F32, tag=f"proj{mt % 2}")
                for k in range(10):
                    nc.tensor.matmul(ps[:mrows, :],
                                     w2[:, k, mt * 128: mt * 128 + mrows],
                                     ff[:, k, :], start=(k == 0), stop=False)
                nc.tensor.matmul(ps[:mrows, :],
                                 w2[0:1, 10, mt * 128: mt * 128 + mrows],
                                 ff[0:1, 10, :], start=False, stop=True)
                cols = slice(tok0, tok0 + 512)
                nc.vector.tensor_tensor(out=self.xT[:mrows, mt, cols],
                                        in0=ps[:mrows, :],
                                        in1=self.xT[:mrows, mt, cols], op=ALU.add)


# ----------------------------------------------------------------------------
# host entry point
# ----------------------------------------------------------------------------

_nc_cache = {}


def _get_nc(nwin=16, chunk_win=8, stages=('a1', 't1', 'a2', 't2', 'ff')):
    key = (nwin, chunk_win, stages)
    if key not in _nc_cache:
        _nc_cache[key] = Builder(nwin, chunk_win, stages).build(
            num_devices=NCORES)
    return _nc_cache[key]


def make_in_maps(inputs, nwin=16):
    x = np.asarray(inputs['x'], np.float32)
    context = np.asarray(inputs['context'], np.float32)
    wd = {k: np.asarray(v, np.float32) for k, v in inputs.items()
          if k not in ('x', 'context')}
    wt = prep_weights(wd)
    shards = shard_x(x, nwin)
    ncore = shards.shape[0]
    in_maps = []
    for c in range(ncore):
        bidx = (c * nwin) // (NH * NH)
        ctxT = _cmajor(np.ascontiguousarray(context[bidx].T), 768)  # [128,6,77]
        m = {'xT': np.ascontiguousarray(_cmajor(shards[c], 384)).astype(bfdt),
             'ctxT': np.ascontiguousarray(ctxT, dtype=np.float32)}
        m.update(wt)
        in_maps.append(m)
    return in_maps


def kernel(**inputs):
    nwin = 16
    nc = _get_nc(nwin)
    in_maps = make_in_maps(inputs, nwin)
    res = run_bass_kernel_spmd(nc, in_maps, list(range(NCORES)))
    outs = np.stack([r['yT'] for r in res.results])  # [8, 128, 3, ntok]
    # undo c-major padding: [8, 128, 3, ntok] -> [8, 384, ntok] -> [8, 320, ntok]
    outs = outs.transpose(0, 2, 1, 3).reshape(NCORES, 384, nwin * SEQ_TOK)[:, :D]
    return unshard_x(outs, nwin).astype(np.float32)

